# revision 10
# baseline (speedup 1.0000x reference)
"""Trainium2 Bass kernel for nn_DAWN_88124138979393 (moe_routing).

Sharding (8 NeuronCores, SPMD — identical instruction stream per core):
  - Token-parallel LN1 / feature / restore / W_O / LN2 / knowledge stages:
    core c owns tokens [c*512, (c+1)*512) of the flattened [4096, 1024]
    residual stream.
  - Head-parallel causal attention: Q^T/K^T/V are exchanged with three
    AllToAlls so core c holds heads {2c, 2c+1} for ALL tokens; the
    normalized attention output returns via a fourth AllToAll.  This keeps
    the causal loop structure identical on every core.
  - All matmuls run in bf16 (PSUM accumulation fp32).  LN statistics,
    softmax normalization and the residual adds stay fp32.
  - gamma1/gamma2 fold into the feature weights on the host; beta1/beta2
    enter as a rank-1 correction (beta @ F) via a K=1 matmul.
  - Softmax skips max-subtraction (scores are O(1e-3) here; exp cannot
    overflow) and defers normalization: PV accumulates unnormalized
    exp-weights, a ones-column appended to V yields Z in the same matmul,
    and 1/Z is applied per-token after transposing back to token-major.
"""
import sys

sys.path.insert(0, '/opt/trn_rl_repo')

import numpy as np

import bass_rust as _bass_rust
import concourse.bass as bass
import concourse.mybir as mybir
from concourse import tile
from concourse.bass_utils import run_bass_kernel_spmd

dt = mybir.dt
AF = mybir.ActivationFunctionType
ALU = mybir.AluOpType
BF = dt.bfloat16
F32 = dt.float32

B, S, D, H, R, N, RK = 2, 2048, 1024, 16, 64, 32, 128
DH = D // H          # 64
T = B * S            # 4096
NC = 8               # cores
TC = T // NC         # 512 tokens per core
NT = TC // 128       # 4 token tiles per core
NRC = (N * R) // 128   # 16 chunks in the qk/v pools (2 neurons per chunk)
NKC = (N * RK) // 128  # 32 chunks in the knowledge pool (1 neuron per chunk)
DC = D // 128        # 8 d-chunks
EPS = 1e-5

MAX_WAITS_PER_INST = 1


# ---------------------------------------------------------------------------
# Tile tail-drain patch: walrus in this container rejects instructions that
# carry more than one sync-wait command.  Split the kernel-tail drain into
# one drain per proc, and post-split every instruction's waits onto NOPs.
# ---------------------------------------------------------------------------

def _split_drain_and_barrier(self, tick_clock, wait_clock):
    gc = tick_clock.global_clock
    ticks = list(gc)
    procs = [i for i, t in enumerate(ticks) if t > 0]
    for g in range(0, max(len(procs), 1), MAX_WAITS_PER_INST):
        group = procs[g:g + MAX_WAITS_PER_INST]
        sub = _bass_rust.VectorClock()
        for i in group:
            sub.require_at_least(i, ticks[i])
        drain_inst = self.nc.sync.drain()
        wait_clock.add_sem_waits(
            drain_inst.ins, _bass_rust.ScopedClock({None: sub})
        )
    self.nc.all_engine_barrier()
    assert self.sems is not None
    popped = self.nc._tile_sem_poison_stack.pop()
    assert popped is self._sem_poison
    self.nc.clear_and_free_semaphores(list(self.sems.allocated().values()))
    self.nc.all_engine_barrier()


tile.TileContext._drain_and_barrier = _split_drain_and_barrier


def split_waits(nc, max_waits=MAX_WAITS_PER_INST):
    counter = [0]
    for f in nc.m.functions:
        for blk in f.blocks:
            i = 0
            while i < len(blk.instructions):
                inst = blk.instructions[i]
                si = inst.sync_info
                if si is not None and len(si.on_wait) > max_waits:
                    waits = list(si.on_wait)
                    si.on_wait = waits[:max_waits]
                    extra = waits[max_waits:]
                    for g in range(0, len(extra), max_waits):
                        nop = _bass_rust.InstNoOp(
                            name=f"WSPLIT-{counter[0]}", ins=[], outs=[])
                        counter[0] += 1
                        nop.engine = inst.engine
                        nop.sync_info = mybir.SyncInfo(
                            on_wait=extra[g:g + max_waits], on_update=[])
                        nc.register_instruction(nop, overwrite=True)
                        blk.instructions.insert(i, nop)
                        i += 1
                i += 1
    return nc


# ---------------------------------------------------------------------------
# Kernel builder
# ---------------------------------------------------------------------------

def build_kernel():
    nc = bass.Bass()

    x_sh = nc.declare_dram_parameter("x_sh", [TC, D], F32, isOutput=False)
    FQK = nc.declare_dram_parameter("FQK", [D, N * R], BF, isOutput=False)
    FV = nc.declare_dram_parameter("FV", [D, N * R], BF, isOutput=False)
    RQK = nc.declare_dram_parameter("RQK", [N * R, D], BF, isOutput=False)
    RV = nc.declare_dram_parameter("RV", [N * R, D], BF, isOutput=False)
    FKN = nc.declare_dram_parameter("FKN", [D, N * RK], BF, isOutput=False)
    RKN = nc.declare_dram_parameter("RKN", [N * RK, D], BF, isOutput=False)
    WOT = nc.declare_dram_parameter("WOT", [D, D], BF, isOutput=False)
    bv_qk = nc.declare_dram_parameter("bv_qk", [1, N * R], BF, isOutput=False)
    bv_v = nc.declare_dram_parameter("bv_v", [1, N * R], BF, isOutput=False)
    bv_kn = nc.declare_dram_parameter("bv_kn", [1, N * RK], BF, isOutput=False)
    wT = {}
    for nm in ("wfq", "wfk", "wfv", "wrq", "wrk", "wrv", "wkf", "wkr"):
        wT[nm] = nc.declare_dram_parameter(nm, [N, TC], BF, isOutput=False)
    ones1 = nc.declare_dram_parameter("ones1", [1, 128], BF, isOutput=False)
    onesr = nc.declare_dram_parameter("onesr", [1, TC], BF, isOutput=False)
    sel64 = nc.declare_dram_parameter("sel64", [128, 64], BF, isOutput=False)
    selfeat = nc.declare_dram_parameter("selfeat", [NRC, N, 128], BF, isOutput=False)
    selkn = nc.declare_dram_parameter("selkn", [NKC, N, 128], BF, isOutput=False)
    identb = nc.declare_dram_parameter("identb", [128, 128], BF, isOutput=False)
    mask0 = nc.declare_dram_parameter("mask0", [128, 256], BF, isOutput=False)
    mask1 = nc.declare_dram_parameter("mask1", [128, 256], BF, isOutput=False)
    onesv = nc.declare_dram_parameter("onesv", [128, 2], BF, isOutput=False)

    y_sh = nc.declare_dram_parameter("y_sh", [TC, D], F32, isOutput=True)

    groups = [list(range(NC))]

    with tile.TileContext(nc) as tc:
        with (
            tc.tile_pool(name="const", bufs=1) as cpool,
            tc.tile_pool(name="persist", bufs=1) as pp,
            tc.tile_pool(name="wstream", bufs=3) as wp,
            tc.tile_pool(name="chunk", bufs=3) as ch,
            tc.tile_pool(name="dram", bufs=1, space="DRAM") as dram,
        ):
            # ---- constants ---------------------------------------------
            ones1_t = cpool.tile([1, 128], BF, name="ones1_t")
            nc.sync.dma_start(ones1_t[:], ones1[:])
            onesr_t = cpool.tile([1, TC], BF, name="onesr_t")
            nc.sync.dma_start(onesr_t[:], onesr[:])
            selfeat_t = [cpool.tile([N, 128], BF, name=f"selfeat{c}")
                         for c in range(NRC)]
            for c in range(NRC):
                nc.sync.dma_start(selfeat_t[c][:], selfeat[c])
            selkn_t = [cpool.tile([N, 128], BF, name=f"selkn{c}")
                       for c in range(NKC)]
            for c in range(NKC):
                nc.sync.dma_start(selkn_t[c][:], selkn[c])
            sel64_t = cpool.tile([128, 64], BF, name="sel64_t")
            nc.sync.dma_start(sel64_t[:], sel64[:])
            ident_t = cpool.tile([128, 128], BF, name="ident_t")
            nc.sync.dma_start(ident_t[:], identb[:])
            mask_t = [cpool.tile([128, 256], BF, name=f"mask_t{i}")
                      for i in range(2)]
            nc.sync.dma_start(mask_t[0][:], mask0[:])
            nc.sync.dma_start(mask_t[1][:], mask1[:])
            onesv_t = cpool.tile([128, 2], BF, name="onesv_t")
            nc.sync.dma_start(onesv_t[:], onesv[:])
            eps_t = cpool.tile([128, 1], F32, name="eps_t")
            nc.gpsimd.memset(eps_t[:], EPS)
            bvqk_t = cpool.tile([1, N * R], BF, name="bvqk_t")
            nc.sync.dma_start(bvqk_t[:], bv_qk[:])
            bvv_t = cpool.tile([1, N * R], BF, name="bvv_t")
            nc.sync.dma_start(bvv_t[:], bv_v[:])
            bvkn_t = cpool.tile([1, N * RK], BF, name="bvkn_t")
            nc.sync.dma_start(bvkn_t[:], bv_kn[:])
            wt_t = {}
            for nm in wT:
                wt_t[nm] = cpool.tile([N, TC], BF, name=f"wt_{nm}")
                nc.sync.dma_start(wt_t[nm][:], wT[nm][:])

            x_t = [pp.tile([128, D], F32, name=f"x_t{i}") for i in range(NT)]
            for i in range(NT):
                nc.sync.dma_start(x_t[i][:], x_sh[i * 128:(i + 1) * 128, :])

            def layernorm_zT(xtiles, zT_tiles, pool, tag):
                for i in range(NT):
                    mu = ch.tile([128, 1], F32, tag="mu")
                    nc.vector.reduce_sum(mu[:], xtiles[i][:],
                                         axis=mybir.AxisListType.X)
                    nc.vector.tensor_scalar_mul(mu[:], mu[:], 1.0 / D)
                    cen = ch.tile([128, D], F32, tag="cen", bufs=2)
                    nc.vector.tensor_scalar(cen[:], xtiles[i][:], mu[:],
                                            None, ALU.subtract)
                    sq = ch.tile([128, D], F32, tag="cen", bufs=2)
                    var = ch.tile([128, 1], F32, tag="var")
                    nc.scalar.activation(sq[:], cen[:], AF.Square,
                                         accum_out=var[:])
                    sd = ch.tile([128, 1], F32, tag="sd")
                    nc.scalar.activation(sd[:], var[:], AF.Sqrt,
                                         scale=1.0 / D, bias=eps_t[:])
                    rsig = ch.tile([128, 1], F32, tag="rsig")
                    nc.vector.reciprocal(rsig[:], sd[:])
                    zb = ch.tile([128, D], BF, tag="zb")
                    nc.vector.tensor_scalar(zb[:], cen[:], rsig[:],
                                            None, ALU.mult)
                    for dc in range(DC):
                        ztp = pool.tile([128, 128], BF, tag=f"ztp{tag}")
                        nc.tensor.transpose(
                            ztp[:], zb[:, dc * 128:(dc + 1) * 128], ident_t[:])
                        nc.scalar.copy(
                            zT_tiles[dc][:, i * 128:(i + 1) * 128], ztp[:])

            zT = [pp.tile([128, TC], BF, name=f"zT{dc}", tag=f"zT{dc}")
                  for dc in range(DC)]
            with tc.tile_pool(name="psLN", bufs=2, space="PSUM") as psLN:
                layernorm_zT(x_t, zT, psLN, "a")

            # ---- feature pools -----------------------------------------
            # G^T group of 4 chunks at a time; weighted sums via PE:
            #   Gs = G^T * bcast(w rows);  h^T += sel.T @ Gs
            def feature_pool(Fdram, bv_t, wh_list, nchunks, psF, tag):
                hs = []
                for wi, _ in enumerate(wh_list):
                    hps = psF.tile([64, TC], F32, tag=f"h{tag}{wi}",
                                   name=f"h{tag}{wi}")
                    hs.append(hps)
                ngrp = nchunks // 4
                for cg in range(ngrp):
                    gtp = [psF.tile([128, TC], F32, tag=f"g{k}",
                                    name=f"g{tag}{cg}_{k}") for k in range(4)]
                    for k in range(4):
                        c = cg * 4 + k
                        nc.tensor.matmul(gtp[k][:],
                                         bv_t[:, c * 128:(c + 1) * 128],
                                         onesr_t[:], start=True, stop=False)
                    for dc in range(DC):
                        fst = wp.tile([128, 512], BF, tag="fs",
                                      name=f"fs{tag}{cg}_{dc}")
                        nc.sync.dma_start(
                            fst[:], Fdram[dc * 128:(dc + 1) * 128,
                                          cg * 512:(cg + 1) * 512])
                        for k in range(4):
                            nc.tensor.matmul(gtp[k][:],
                                             fst[:, k * 128:(k + 1) * 128],
                                             zT[dc][:], start=False,
                                             stop=(dc == DC - 1))
                    for k in range(4):
                        c = cg * 4 + k
                        gcp = ch.tile([128, TC], BF, tag="gcp")
                        nc.scalar.copy(gcp[:], gtp[k][:])
                        for wi, wname in enumerate(wh_list):
                            wb = psF.tile([128, TC], F32, tag="wb", bufs=2)
                            nc.tensor.matmul(wb[:], selfeat_t[c][:],
                                             wt_t[wname][:],
                                             start=True, stop=True)
                            gs = ch.tile([128, TC], BF, tag="gs")
                            nc.vector.tensor_mul(gs[:], gcp[:], wb[:])
                            nc.tensor.matmul(hs[wi][:], sel64_t[:], gs[:],
                                             start=(c == 0),
                                             stop=(c == nchunks - 1))
                return hs

            def hstack(h_ps, name):
                t = pp.tile([128, TC], BF, name=name)
                nc.scalar.copy(t[0:64, :], h_ps[:])
                nc.scalar.copy(t[64:128, :], h_ps[:])
                return t

            with tc.tile_pool(name="psF", bufs=1, space="PSUM") as psF:
                hq_ps, hk_ps = feature_pool(FQK, bvqk_t, ["wfq", "wfk"],
                                            NRC, psF, "qk")
                hq_st = hstack(hq_ps, "hq_st")
                hk_st = hstack(hk_ps, "hk_st")

            with tc.tile_pool(name="psFV", bufs=1, space="PSUM") as psFV:
                (hv_ps,) = feature_pool(FV, bvv_t, ["wfv"], NRC, psFV, "v")
                hv_st = hstack(hv_ps, "hv_st")

            # ---- u chunks + restores -----------------------------------
            u_t = [pp.tile([128, TC], BF, name=f"u_t{c}", tag=f"u{c}")
                   for c in range(NRC)]

            def build_u(wname, h_st, u_tiles, nper, psU):
                for c, ut in enumerate(u_tiles):
                    wb = psU.tile([128, TC], F32, tag="uwb")
                    sel = selfeat_t[c] if nper == 2 else selkn_t[c]
                    nc.tensor.matmul(wb[:], sel[:], wt_t[wname][:],
                                     start=True, stop=True)
                    nc.vector.tensor_mul(ut[:], h_st[:], wb[:])

            def restore_T(u_tiles, Rdram, out_name, psR, ib):
                ps = [psR.tile([128, TC], F32, tag=f"rt{dc}",
                               name=f"{out_name}ps{dc}") for dc in range(DC)]
                for c in range(NRC):
                    rst = wp.tile([128, D], BF, tag="rs",
                                  name=f"{out_name}rs{c}")
                    nc.sync.dma_start(rst[:], Rdram[c * 128:(c + 1) * 128, :])
                    for dc in range(DC):
                        nc.tensor.matmul(ps[dc][:],
                                         rst[:, dc * 128:(dc + 1) * 128],
                                         u_tiles[c][:], start=(c == 0),
                                         stop=(c == NRC - 1))
                for dc in range(DC):
                    ot = ch.tile([128, TC], BF, tag="rT")
                    nc.scalar.copy(ot[:], ps[dc][:])
                    nc.sync.dma_start(ib[dc * 128:(dc + 1) * 128, :], ot[:])

            def a2a_pair(in_shape, out_shape, name):
                ib = dram.tile(in_shape, BF, name=f"{name}_in")
                ob = dram.tile(out_shape, BF, name=f"{name}_out")
                return ib, ob

            qt_ib, qt_ob = a2a_pair([D, TC], [D, TC], "a2aq")
            kt_ib, kt_ob = a2a_pair([D, TC], [D, TC], "a2ak")
            v_ib, v_ob = a2a_pair([NC, TC, 128], [NC, TC, 128], "a2av")

            with tc.tile_pool(name="psU1", bufs=2, space="PSUM") as psU:
                build_u("wrq", hq_st, u_t, 2, psU)
            with tc.tile_pool(name="psQ", bufs=1, space="PSUM") as psQ:
                restore_T(u_t, RQK, "qT", psQ, qt_ib)
            nc.gpsimd.collective_compute(
                "AllToAll", ALU.bypass, replica_groups=groups,
                ins=[qt_ib.opt()], outs=[qt_ob.opt()])

            with tc.tile_pool(name="psU2", bufs=2, space="PSUM") as psU:
                build_u("wrk", hk_st, u_t, 2, psU)
            with tc.tile_pool(name="psK", bufs=1, space="PSUM") as psK:
                restore_T(u_t, RQK, "kT", psK, kt_ib)
            nc.gpsimd.collective_compute(
                "AllToAll", ALU.bypass, replica_groups=groups,
                ins=[kt_ib.opt()], outs=[kt_ob.opt()])

            with tc.tile_pool(name="psU3", bufs=2, space="PSUM") as psU:
                build_u("wrv", hv_st, u_t, 2, psU)
            with tc.tile_pool(name="psV", bufs=1, space="PSUM") as psV:
                vps = [psV.tile([128, 512], F32, tag=f"vps{k}",
                                 name=f"vps{k}") for k in range(NT * 2)]
                for c in range(NRC):
                    rst = wp.tile([128, D], BF, tag="rs", name=f"vrs{c}")
                    nc.sync.dma_start(rst[:], RV[c * 128:(c + 1) * 128, :])
                    for i in range(NT):
                        for j in range(2):
                            nc.tensor.matmul(
                                vps[i * 2 + j][:],
                                u_t[c][:, i * 128:(i + 1) * 128],
                                rst[:, j * 512:(j + 1) * 512],
                                start=(c == 0), stop=(c == NRC - 1))
                for i in range(NT):
                    vsb = ch.tile([128, D], BF, tag="vT", bufs=2)
                    for j in range(2):
                        nc.scalar.copy(vsb[:, j * 512:(j + 1) * 512],
                                       vps[i * 2 + j][:])
                    for p in range(NC):
                        nc.sync.dma_start(v_ib[p, i * 128:(i + 1) * 128, :],
                                          vsb[:, p * 128:(p + 1) * 128])
            nc.gpsimd.collective_compute(
                "AllToAll", ALU.bypass, replica_groups=groups,
                ins=[v_ib.opt()], outs=[v_ob.opt()])

            # ---- attention (heads 2c, 2c+1; all tokens) ----------------
            qTh = pp.tile([128, T], BF, name="qTh")
            kTh = pp.tile([128, T], BF, name="kTh")
            qt_ov = qt_ob.rearrange("(r p) t -> r p t", p=128)
            kt_ov = kt_ob.rearrange("(r p) t -> r p t", p=128)
            for r in range(NC):
                nc.sync.dma_start(qTh[:, r * TC:(r + 1) * TC], qt_ov[r])
                nc.sync.dma_start(kTh[:, r * TC:(r + 1) * TC], kt_ov[r])
            v_ov = v_ob.rearrange("r (i p) e -> (r i) p e", p=128)
            vi_t = []
            for kc in range(T // 128):
                vt = pp.tile([128, 2, 65], BF, name=f"vi{kc}")
                for hp in range(2):
                    nc.sync.dma_start(vt[:, hp, 0:64],
                                      v_ov[kc][:, hp * 64:(hp + 1) * 64])
                nc.vector.tensor_copy(vt[:, :, 64], onesv_t[:])
                vi_t.append(vt)

            attnN = [pp.tile([128, 128], BF, name=f"attnN{i}")
                     for i in range(T // 128)]
            attnT = pp.tile([128, T], BF, name="attnT")

            NQB = S // 256
            with tc.tile_pool(name="psATT", bufs=1, space="PSUM") as psT:
                st_tiles = [psT.tile([128, 256], F32, tag=f"st{k}",
                                     name=f"st{k}") for k in range(3)]
                ot_tiles = [psT.tile([65, 256], F32, tag=f"ot{k}",
                                     name=f"ot{k}") for k in range(2)]
                op_tiles = [psT.tile([128, 128], BF, tag=f"op{k}",
                                     name=f"op{k}") for k in range(2)]
                sti = [0]
                oti = [0]
                opi = [0]

                def next_t(tiles, idx):
                    t = tiles[idx[0] % len(tiles)]
                    idx[0] += 1
                    return t

                for b in range(B):
                    for hp in range(2):
                        for qb in range(NQB):
                            ot = next_t(ot_tiles, oti)
                            q0 = b * S + qb * 256
                            for kb in range(qb + 1):
                                for kc in range(2):
                                    k0 = b * S + kb * 256 + kc * 128
                                    st = next_t(st_tiles, sti)
                                    nc.tensor.matmul(
                                        st[:],
                                        kTh[hp * 64:(hp + 1) * 64, k0:k0 + 128],
                                        qTh[hp * 64:(hp + 1) * 64, q0:q0 + 256],
                                        start=True, stop=True)
                                    es = ch.tile([128, 256], BF, tag="es")
                                    nc.scalar.activation(es[:], st[:], AF.Exp,
                                                         scale=0.125)
                                    if kb == qb:
                                        nc.vector.tensor_mul(es[:], es[:],
                                                             mask_t[kc][:])
                                    nc.tensor.matmul(
                                        ot[:], vi_t[k0 // 128][:, hp, :], es[:],
                                        start=(kb == 0 and kc == 0),
                                        stop=(kb == qb and kc == 1))
                            ots = ch.tile([65, 256], BF, tag="ots")
                            nc.scalar.copy(ots[:], ot[:])
                            for qc in range(2):
                                op = next_t(op_tiles, opi)
                                nc.tensor.transpose(
                                    op[:, 0:65], ots[:, qc * 128:(qc + 1) * 128],
                                    ident_t[0:65, 0:65])
                                oq = ch.tile([128, 65], F32, tag="oq")
                                nc.vector.tensor_copy(oq[:], op[:, 0:65])
                                rz = ch.tile([128, 1], F32, tag="rz")
                                nc.vector.reciprocal(rz[:], oq[:, 64:65])
                                ti = (q0 + qc * 128) // 128
                                nc.vector.tensor_scalar(
                                    attnN[ti][:, hp * 64:(hp + 1) * 64],
                                    oq[:, 0:64], rz[:], None, ALU.mult)

                for i in range(T // 128):
                    op = next_t(op_tiles, opi)
                    nc.tensor.transpose(op[:], attnN[i][:], ident_t[:])
                    nc.scalar.copy(attnT[:, i * 128:(i + 1) * 128], op[:])

            at_ib = dram.tile([NC, 128, TC], BF, name="a2aa_in")
            at_ob = dram.tile([NC, 128, TC], BF, name="a2aa_out")
            for r in range(NC):
                nc.sync.dma_start(at_ib[r], attnT[:, r * TC:(r + 1) * TC])
            nc.gpsimd.collective_compute(
                "AllToAll", ALU.bypass, replica_groups=groups,
                ins=[at_ib.opt()], outs=[at_ob.opt()])

            atT = [pp.tile([128, TC], BF, name=f"atT{r}") for r in range(NC)]
            for r in range(NC):
                nc.sync.dma_start(atT[r][:], at_ob[r])

            # ---- W_O + residual ----------------------------------------
            x1_t = [pp.tile([128, D], F32, name=f"x1_t{i}") for i in range(NT)]
            with tc.tile_pool(name="psWO", bufs=1, space="PSUM") as psW:
                aops = [psW.tile([128, 512], F32, tag=f"ao{k}",
                                 name=f"ao{k}") for k in range(NT * 2)]
                for dc in range(DC):
                    wst = wp.tile([128, D], BF, tag="rs", name=f"wos{dc}")
                    nc.sync.dma_start(wst[:], WOT[dc * 128:(dc + 1) * 128, :])
                    for i in range(NT):
                        for j in range(2):
                            nc.tensor.matmul(
                                aops[i * 2 + j][:],
                                atT[dc][:, i * 128:(i + 1) * 128],
                                wst[:, j * 512:(j + 1) * 512],
                                start=(dc == 0), stop=(dc == DC - 1))
                for i in range(NT):
                    for j in range(2):
                        nc.vector.tensor_add(
                            x1_t[i][:, j * 512:(j + 1) * 512],
                            x_t[i][:, j * 512:(j + 1) * 512],
                            aops[i * 2 + j][:])

            # ---- LN2 + knowledge ---------------------------------------
            z2T = [pp.tile([128, TC], BF, name=f"z2T{dc}", tag=f"zT{dc}")
                   for dc in range(DC)]
            with tc.tile_pool(name="psLN2", bufs=2, space="PSUM") as psLN2:
                layernorm_zT(x1_t, z2T, psLN2, "b")

            h2_sb = pp.tile([128, TC], BF, name="h2_sb")
            with tc.tile_pool(name="psKF", bufs=1, space="PSUM") as psK2:
                h2_ps = psK2.tile([128, TC], F32, tag="h2", name="h2_ps")
                ngrp = NKC // 4
                for cg in range(ngrp):
                    gtp = [psK2.tile([128, TC], F32, tag=f"g{k}",
                                     name=f"g2{cg}_{k}") for k in range(4)]
                    for k in range(4):
                        c = cg * 4 + k
                        nc.tensor.matmul(gtp[k][:],
                                         bvkn_t[:, c * 128:(c + 1) * 128],
                                         onesr_t[:], start=True, stop=False)
                    for dc in range(DC):
                        fst = wp.tile([128, 512], BF, tag="fs",
                                      name=f"fkn{cg}_{dc}")
                        nc.sync.dma_start(
                            fst[:], FKN[dc * 128:(dc + 1) * 128,
                                        cg * 512:(cg + 1) * 512])
                        for k in range(4):
                            nc.tensor.matmul(gtp[k][:],
                                             fst[:, k * 128:(k + 1) * 128],
                                             z2T[dc][:], start=False,
                                             stop=(dc == DC - 1))
                    for k in range(4):
                        c = cg * 4 + k
                        gcp = ch.tile([128, TC], BF, tag="gcp")
                        nc.scalar.copy(gcp[:], gtp[k][:])
                        wb = psK2.tile([128, TC], F32, tag="wb", bufs=2)
                        nc.tensor.matmul(wb[:], selkn_t[c][:],
                                         wt_t["wkf"][:],
                                         start=True, stop=True)
                        gs = ch.tile([128, TC], BF, tag="gs")
                        nc.vector.tensor_mul(gs[:], gcp[:], wb[:])
                        nc.tensor.matmul(h2_ps[:], ident_t[:], gs[:],
                                         start=(c == 0), stop=(c == NKC - 1))
                nc.scalar.copy(h2_sb[:], h2_ps[:])

            u2_t = [pp.tile([128, TC], BF, name=f"u2_t{c}",
                            tag=(f"u{c}" if c < NRC else f"u2{c}"))
                    for c in range(NKC)]
            with tc.tile_pool(name="psU4", bufs=2, space="PSUM") as psU:
                build_u("wkr", h2_sb, u2_t, 1, psU)

            with tc.tile_pool(name="psKR", bufs=1, space="PSUM") as psKR:
                kps = [psKR.tile([128, 512], F32, tag=f"kp{k}",
                                 name=f"kp{k}") for k in range(NT * 2)]
                for c in range(NKC):
                    rst = wp.tile([128, D], BF, tag="rs", name=f"krs{c}")
                    nc.sync.dma_start(rst[:], RKN[c * 128:(c + 1) * 128, :])
                    for i in range(NT):
                        for j in range(2):
                            nc.tensor.matmul(
                                kps[i * 2 + j][:],
                                u2_t[c][:, i * 128:(i + 1) * 128],
                                rst[:, j * 512:(j + 1) * 512],
                                start=(c == 0), stop=(c == NKC - 1))
                for i in range(NT):
                    yf = ch.tile([128, D], F32, tag="cen", bufs=2)
                    for j in range(2):
                        nc.vector.tensor_add(
                            yf[:, j * 512:(j + 1) * 512],
                            x1_t[i][:, j * 512:(j + 1) * 512],
                            kps[i * 2 + j][:])
                    nc.sync.dma_start(y_sh[i * 128:(i + 1) * 128, :], yf[:])

    split_waits(nc)
    return nc


# ---------------------------------------------------------------------------
# Host side
# ---------------------------------------------------------------------------

_NC_CACHE = {}


def _get_nc():
    if "nc" not in _NC_CACHE:
        _NC_CACHE["nc"] = build_kernel()
    return _NC_CACHE["nc"]


def _bf16(a):
    import ml_dtypes
    return np.ascontiguousarray(
        np.asarray(a, dtype=np.float32)).astype(ml_dtypes.bfloat16)


def _selfeat():
    m = np.zeros((NRC, N, 128), np.float32)
    for c in range(NRC):
        m[c, 2 * c, 0:64] = 1.0
        m[c, 2 * c + 1, 64:128] = 1.0
    return m


def _selkn():
    m = np.zeros((NKC, N, 128), np.float32)
    for c in range(NKC):
        m[c, c, :] = 1.0
    return m


def prepare_inputs(x, f_qk, f_v, r_qk, r_v, f_know, r_know, W_O,
                   gamma1, beta1, gamma2, beta2,
                   w_fq, w_fk, w_fv, w_rq, w_rk, w_rv, w_know_f, w_know_r):
    x = np.asarray(x, np.float32).reshape(T, D)
    gamma1 = np.asarray(gamma1, np.float32)
    beta1 = np.asarray(beta1, np.float32)
    gamma2 = np.asarray(gamma2, np.float32)
    beta2 = np.asarray(beta2, np.float32)

    FQKh = (np.asarray(f_qk, np.float32) * gamma1[None, :, None]) \
        .transpose(1, 0, 2).reshape(D, N * R)
    FVh = (np.asarray(f_v, np.float32) * gamma1[None, :, None]) \
        .transpose(1, 0, 2).reshape(D, N * R)
    FKNh = (np.asarray(f_know, np.float32) * gamma2[None, :, None]) \
        .transpose(1, 0, 2).reshape(D, N * RK)
    RQKh = np.asarray(r_qk, np.float32).reshape(N * R, D)
    RVh = np.asarray(r_v, np.float32).reshape(N * R, D)
    RKNh = np.asarray(r_know, np.float32).reshape(N * RK, D)
    WOTh = np.ascontiguousarray(np.asarray(W_O, np.float32).T)

    qi = np.arange(256)[None, :]
    ki = np.arange(128)[:, None]
    shared = {
        "FQK": _bf16(FQKh), "FV": _bf16(FVh), "RQK": _bf16(RQKh),
        "RV": _bf16(RVh), "FKN": _bf16(FKNh), "RKN": _bf16(RKNh),
        "WOT": _bf16(WOTh),
        "bv_qk": _bf16((beta1 @ FQKh)[None, :]),
        "bv_v": _bf16((beta1 @ FVh)[None, :]),
        "bv_kn": _bf16((beta2 @ FKNh)[None, :]),
        "ones1": _bf16(np.ones((1, 128))),
        "onesr": _bf16(np.ones((1, TC))),
        "sel64": _bf16(np.vstack([np.eye(64), np.eye(64)])),
        "selfeat": _bf16(_selfeat()),
        "selkn": _bf16(_selkn()),
        "identb": _bf16(np.eye(128)),
        "mask0": _bf16((qi >= ki).astype(np.float32)),
        "mask1": _bf16((qi >= ki + 128).astype(np.float32)),
        "onesv": _bf16(np.ones((128, 2))),
    }

    wmap = {"wfq": w_fq, "wfk": w_fk, "wfv": w_fv, "wrq": w_rq,
            "wrk": w_rk, "wrv": w_rv, "wkf": w_know_f, "wkr": w_know_r}
    in_maps = []
    for c in range(NC):
        m = dict(shared)
        m["x_sh"] = np.ascontiguousarray(x[c * TC:(c + 1) * TC])
        for nm, w in wmap.items():
            wt = np.asarray(w, np.float32).reshape(T, N)[c * TC:(c + 1) * TC].T
            m[nm] = _bf16(wt)
        in_maps.append(m)
    return in_maps


def assemble_output(results):
    out = np.empty((T, D), np.float32)
    for c in range(NC):
        out[c * TC:(c + 1) * TC] = results[c]["y_sh"]
    return out.reshape(B, S, D)


def kernel(**inputs):
    nc = _get_nc()
    in_maps = prepare_inputs(**inputs)
    res = run_bass_kernel_spmd(nc, in_maps, list(range(NC)))
    return assemble_output(res.results)


if __name__ == "__main__":
    build_kernel()
    print("kernel built OK")


# revision 22
# speedup vs baseline: 1.1195x; 1.1195x over previous
"""Trainium2 Bass kernel for nn_DAWN_88124138979393 (moe_routing).

Sharding (8 NeuronCores, SPMD — identical instruction stream per core):
  - Token-parallel LN1 / feature / restore / W_O / LN2 / knowledge stages:
    core c owns tokens [c*512, (c+1)*512) of the flattened [4096, 1024]
    residual stream.
  - Head-parallel causal attention: Q^T/K^T/V are exchanged with three
    AllToAlls so core c holds heads {2c, 2c+1} for ALL tokens; the
    normalized attention output returns via a fourth AllToAll.  This keeps
    the causal loop structure identical on every core.
  - All matmuls run in bf16 (PSUM accumulation fp32).  LN statistics,
    softmax normalization and the residual adds stay fp32.
  - gamma1/gamma2 fold into the feature weights on the host; nonzero
    beta1/beta2 enter as a rank-1 correction (beta @ F) via K=1 matmuls
    (emitted only when some beta is nonzero — they are zeros here).
  - Softmax skips max-subtraction (scores are O(1e-3) here; exp cannot
    overflow) and defers normalization: PV accumulates unnormalized
    exp-weights, a ones-column appended to V yields Z in the same matmul,
    and 1/Z is applied per-token after transposing back to token-major.
  - Weights ship in chunk-major host layouts so each weight matrix arrives
    in O(1) large DMAs (per-DMA fixed overhead dominates otherwise).
"""
import sys

sys.path.insert(0, '/opt/trn_rl_repo')

import numpy as np

import bass_rust as _bass_rust
import concourse.bass as bass
import concourse.mybir as mybir
from concourse import tile
from concourse.bass_utils import run_bass_kernel_spmd

dt = mybir.dt
AF = mybir.ActivationFunctionType
ALU = mybir.AluOpType
BF = dt.bfloat16
F32 = dt.float32

B, S, D, H, R, N, RK = 2, 2048, 1024, 16, 64, 32, 128
DH = D // H          # 64
T = B * S            # 4096
NC = 8               # cores
TC = T // NC         # 512 tokens per core
NT = TC // 128       # 4 token tiles per core
NRC = (N * R) // 128   # 16 chunks in the qk/v pools (2 neurons per chunk)
NKC = (N * RK) // 128  # 32 chunks in the knowledge pool (1 neuron per chunk)
DC = D // 128        # 8 d-chunks
EPS = 1e-5

MAX_WAITS_PER_INST = 1

# const blob column offsets (bf16 [128, 768])
CB_IDENT = 0      # [128, 128] identity
CB_SEL64 = 128    # [128, 64]  I64 stacked twice
CB_MASK0 = 192    # [128, 256] causal mask, diag chunk 0
CB_MASK1 = 448    # [128, 256] causal mask, diag chunk 1
CB_ONES = 704     # [128, 64]  ones (V-interleave Z column fill)
CB_COLS = 768

W_ORDER = ("wfq", "wfk", "wfv", "wrq", "wrk", "wrv", "wkf", "wkr")


# ---------------------------------------------------------------------------
# Tile tail-drain patch: walrus in this container rejects instructions that
# carry more than one sync-wait command.  Split the kernel-tail drain into
# one drain per proc, and post-split every instruction's waits onto NOPs.
# ---------------------------------------------------------------------------

def _split_drain_and_barrier(self, tick_clock, wait_clock):
    gc = tick_clock.global_clock
    ticks = list(gc)
    procs = [i for i, t in enumerate(ticks) if t > 0]
    for g in range(0, max(len(procs), 1), MAX_WAITS_PER_INST):
        group = procs[g:g + MAX_WAITS_PER_INST]
        sub = _bass_rust.VectorClock()
        for i in group:
            sub.require_at_least(i, ticks[i])
        drain_inst = self.nc.sync.drain()
        wait_clock.add_sem_waits(
            drain_inst.ins, _bass_rust.ScopedClock({None: sub})
        )
    self.nc.all_engine_barrier()
    assert self.sems is not None
    popped = self.nc._tile_sem_poison_stack.pop()
    assert popped is self._sem_poison
    self.nc.clear_and_free_semaphores(list(self.sems.allocated().values()))
    self.nc.all_engine_barrier()


tile.TileContext._drain_and_barrier = _split_drain_and_barrier


def split_waits(nc, max_waits=MAX_WAITS_PER_INST):
    counter = [0]
    for f in nc.m.functions:
        for blk in f.blocks:
            i = 0
            while i < len(blk.instructions):
                inst = blk.instructions[i]
                si = inst.sync_info
                if si is not None and len(si.on_wait) > max_waits:
                    waits = list(si.on_wait)
                    si.on_wait = waits[:max_waits]
                    extra = waits[max_waits:]
                    for g in range(0, len(extra), max_waits):
                        nop = _bass_rust.InstNoOp(
                            name=f"WSPLIT-{counter[0]}", ins=[], outs=[])
                        counter[0] += 1
                        nop.engine = inst.engine
                        nop.sync_info = mybir.SyncInfo(
                            on_wait=extra[g:g + max_waits], on_update=[])
                        nc.register_instruction(nop, overwrite=True)
                        blk.instructions.insert(i, nop)
                        i += 1
                i += 1
    return nc


# ---------------------------------------------------------------------------
# Kernel builder
# ---------------------------------------------------------------------------

def build_kernel(with_bv=False):
    nc = bass.Bass()

    x_sh = nc.declare_dram_parameter("x_sh", [TC, D], F32, isOutput=False)
    # chunk-major weight layouts (see prepare_inputs)
    FQK = nc.declare_dram_parameter("FQK", [128, DC, N * R], BF, isOutput=False)
    FV = nc.declare_dram_parameter("FV", [128, DC, N * R], BF, isOutput=False)
    FKN = nc.declare_dram_parameter("FKN", [128, DC, N * RK], BF, isOutput=False)
    RQK = nc.declare_dram_parameter("RQK", [128, NRC, D], BF, isOutput=False)
    RV = nc.declare_dram_parameter("RV", [128, NRC, D], BF, isOutput=False)
    RKN = nc.declare_dram_parameter("RKN", [128, NKC, D], BF, isOutput=False)
    WOT = nc.declare_dram_parameter("WOT", [128, DC, D], BF, isOutput=False)
    wts = nc.declare_dram_parameter("wts", [N, len(W_ORDER), TC], BF,
                                    isOutput=False)
    selfeat = nc.declare_dram_parameter("selfeat", [N, NRC, 128], BF,
                                        isOutput=False)
    selkn = nc.declare_dram_parameter("selkn", [N, NKC, 128], BF,
                                      isOutput=False)
    cblob = nc.declare_dram_parameter("cblob", [128, CB_COLS], BF,
                                      isOutput=False)
    oblob = nc.declare_dram_parameter("oblob", [1, 128 + TC], BF,
                                      isOutput=False)
    if with_bv:
        bvb = nc.declare_dram_parameter(
            "bvb", [1, 2 * N * R + N * RK], BF, isOutput=False)

    y_sh = nc.declare_dram_parameter("y_sh", [TC, D], F32, isOutput=True)

    groups = [list(range(NC))]

    with tile.TileContext(nc) as tc:
        with (
            tc.tile_pool(name="const", bufs=1) as cpool,
            tc.tile_pool(name="persist", bufs=1) as pp,
            tc.tile_pool(name="wfam", bufs=1) as wf,
            tc.tile_pool(name="chunk", bufs=3) as ch,
            tc.tile_pool(name="dram", bufs=1, space="DRAM") as dram,
        ):
            # ---- constants (a handful of blob DMAs) --------------------
            cb = cpool.tile([128, CB_COLS], BF, name="cb")
            nc.sync.dma_start(cb[:], cblob[:])
            ident_t = cb[:, CB_IDENT:CB_IDENT + 128]
            sel64_t = cb[:, CB_SEL64:CB_SEL64 + 64]
            mask_t = [cb[:, CB_MASK0:CB_MASK0 + 256],
                      cb[:, CB_MASK1:CB_MASK1 + 256]]
            ones64_t = cb[:, CB_ONES:CB_ONES + 64]
            ob = cpool.tile([1, 128 + TC], BF, name="ob")
            nc.sync.dma_start(ob[:], oblob[:])
            onesr_t = ob[:, 128:128 + TC]
            eps_t = cpool.tile([128, 1], F32, name="eps_t")
            nc.gpsimd.memset(eps_t[:], EPS)
            wt_all = cpool.tile([N, len(W_ORDER), TC], BF, name="wt_all")
            nc.sync.dma_start(wt_all[:], wts[:])
            wt_t = {nm: wt_all[:, wi, :] for wi, nm in enumerate(W_ORDER)}
            selfeat_t = cpool.tile([N, NRC, 128], BF, name="selfeat_t")
            nc.sync.dma_start(selfeat_t[:], selfeat[:])

            if with_bv:
                bvb_t = cpool.tile([1, 2 * N * R + N * RK], BF, name="bvb_t")
                nc.sync.dma_start(bvb_t[:], bvb[:])
                bv_of = {"qk": 0, "v": N * R, "kn": 2 * N * R}

            x_t = [pp.tile([128, D], F32, name=f"x_t{i}") for i in range(NT)]
            for i in range(NT):
                nc.sync.dma_start(x_t[i][:], x_sh[i * 128:(i + 1) * 128, :])

            def layernorm_zT(xtiles, zT_tiles, pool, tag):
                for i in range(NT):
                    mu = ch.tile([128, 1], F32, tag="mu")
                    nc.vector.reduce_sum(mu[:], xtiles[i][:],
                                         axis=mybir.AxisListType.X)
                    nc.vector.tensor_scalar_mul(mu[:], mu[:], 1.0 / D)
                    cen = ch.tile([128, D], F32, tag="cen", bufs=2)
                    nc.vector.tensor_scalar(cen[:], xtiles[i][:], mu[:],
                                            None, ALU.subtract)
                    sq = ch.tile([128, D], F32, tag="cen", bufs=2)
                    var = ch.tile([128, 1], F32, tag="var")
                    nc.scalar.activation(sq[:], cen[:], AF.Square,
                                         accum_out=var[:])
                    sd = ch.tile([128, 1], F32, tag="sd")
                    nc.scalar.activation(sd[:], var[:], AF.Sqrt,
                                         scale=1.0 / D, bias=eps_t[:])
                    rsig = ch.tile([128, 1], F32, tag="rsig")
                    nc.vector.reciprocal(rsig[:], sd[:])
                    zb = ch.tile([128, D], BF, tag="zb")
                    nc.vector.tensor_scalar(zb[:], cen[:], rsig[:],
                                            None, ALU.mult)
                    for dc in range(DC):
                        ztp = pool.tile([128, 128], BF, tag=f"ztp{tag}")
                        nc.tensor.transpose(
                            ztp[:], zb[:, dc * 128:(dc + 1) * 128], ident_t)
                        nc.scalar.copy(
                            zT_tiles[dc][:, i * 128:(i + 1) * 128], ztp[:])

            zT = [pp.tile([128, TC], BF, name=f"zT{dc}", tag=f"zT{dc}")
                  for dc in range(DC)]
            with tc.tile_pool(name="psLN", bufs=2, space="PSUM") as psLN:
                layernorm_zT(x_t, zT, psLN, "a")

            # ---- feature pools -----------------------------------------
            def feature_half(Fdram, half, bvkey, wh_list, nchunks, psF, hs,
                             tag, pending):
                # pending: list of deferred (h_ps, gs, c) SEL matmuls from
                # the previous chunk group — emitted during this group's
                # gt accumulation so the PE never stalls on the DVE muls.
                fam = wf.tile([128, DC, 1024], BF, tag="Ffam",
                              name=f"F{tag}{half}")
                nc.sync.dma_start(
                    fam[:], Fdram[:, :, half * 1024:(half + 1) * 1024])

                def flush_pending():
                    for h_ps, gs, c in pending:
                        nc.tensor.matmul(h_ps[:], sel64_t, gs[:],
                                         start=(c == 0),
                                         stop=(c == nchunks - 1))
                    pending.clear()

                for cg in range(2):        # 2 groups of 4 chunks per half
                    gtp = [psF.tile([128, TC], F32, tag=f"g{k}",
                                    name=f"g{tag}{half}{cg}_{k}")
                           for k in range(4)]
                    for k in range(4):
                        c = (half * 2 + cg) * 4 + k
                        if with_bv:
                            nc.tensor.matmul(
                                gtp[k][:],
                                bvb_t[:, bv_of[bvkey] + c * 128:
                                      bv_of[bvkey] + (c + 1) * 128],
                                onesr_t, start=True, stop=False)
                    for dc in range(DC):
                        for k in range(4):
                            ci = cg * 4 + k
                            nc.tensor.matmul(
                                gtp[k][:],
                                fam[:, dc, ci * 128:(ci + 1) * 128],
                                zT[dc][:],
                                start=(not with_bv and dc == 0),
                                stop=(dc == DC - 1))
                        if dc == 0:
                            flush_pending()
                    for k in range(4):
                        c = (half * 2 + cg) * 4 + k
                        gcp = ch.tile([128, TC], BF, tag="gcp", bufs=4)
                        nc.scalar.copy(gcp[:], gtp[k][:])
                        for wi, wname in enumerate(wh_list):
                            wb = psF.tile([128, TC], F32, tag="wb", bufs=2)
                            nc.tensor.matmul(wb[:], selfeat_t[:, c, :],
                                             wt_t[wname],
                                             start=True, stop=True)
                            gs = ch.tile([128, TC], BF, tag="gs", bufs=8)
                            nc.vector.tensor_mul(gs[:], gcp[:], wb[:])
                            pending.append((hs[wi], gs, c))

            def hstack(h_ps, name):
                t = pp.tile([128, TC], BF, name=name)
                nc.scalar.copy(t[0:64, :], h_ps[:])
                nc.scalar.copy(t[64:128, :], h_ps[:])
                return t

            with tc.tile_pool(name="psF", bufs=1, space="PSUM") as psF:
                hq_ps = psF.tile([64, TC], F32, tag="hq", name="hq_ps")
                hk_ps = psF.tile([64, TC], F32, tag="hk", name="hk_ps")
                pend = []
                for half in range(2):
                    feature_half(FQK, half, "qk", ["wfq", "wfk"], NRC, psF,
                                 [hq_ps, hk_ps], "qk", pend)
                for h_ps, gs, c in pend:
                    nc.tensor.matmul(h_ps[:], sel64_t, gs[:],
                                     start=(c == 0), stop=(c == NRC - 1))
                pend.clear()
                hq_st = hstack(hq_ps, "hq_st")
                hk_st = hstack(hk_ps, "hk_st")
                hv_ps = psF.tile([64, TC], F32, tag="hq", name="hv_ps")
                for half in range(2):
                    feature_half(FV, half, "v", ["wfv"], NRC, psF,
                                 [hv_ps], "v", pend)
                for h_ps, gs, c in pend:
                    nc.tensor.matmul(h_ps[:], sel64_t, gs[:],
                                     start=(c == 0), stop=(c == NRC - 1))
                pend.clear()
                hv_st = hstack(hv_ps, "hv_st")

            # ---- u chunks + restores -----------------------------------
            u_t = [pp.tile([128, TC], BF, name=f"u_t{c}", tag=f"u{c}")
                   for c in range(NRC)]

            def build_u_chunk(wname, h_st, ut, c, seln, psU, tag="uwb",
                              nbufs=2):
                wb = psU.tile([128, TC], F32, tag=tag, bufs=nbufs)
                nc.tensor.matmul(wb[:], seln[:, c, :], wt_t[wname],
                                 start=True, stop=True)
                nc.vector.tensor_mul(ut[:], h_st[:], wb[:])

            def restore_T(u_tiles, Rfam, out_name, psR, ib,
                          uspec=None):
                # two passes of 4 psum banks; pass 0 builds u on the fly
                for hp in range(2):
                    ps = [psR.tile([128, TC], F32, tag=f"rt{k}",
                                   name=f"{out_name}ps{hp}_{k}")
                          for k in range(4)]
                    for c in range(NRC):
                        if hp == 0 and uspec is not None:
                            build_u_chunk(uspec[0], uspec[1], u_tiles[c],
                                          c, uspec[2], psR)
                        for k in range(4):
                            dc = hp * 4 + k
                            nc.tensor.matmul(
                                ps[k][:],
                                Rfam[:, c, dc * 128:(dc + 1) * 128],
                                u_tiles[c][:], start=(c == 0),
                                stop=(c == NRC - 1))
                    for k in range(4):
                        dc = hp * 4 + k
                        ot = ch.tile([128, TC], BF, tag="rT")
                        nc.scalar.copy(ot[:], ps[k][:])
                        nc.sync.dma_start(ib[dc * 128:(dc + 1) * 128, :],
                                          ot[:])

            qt_ib = dram.tile([D, TC], BF, name="a2aq_in")
            qt_ob = dram.tile([D, TC], BF, name="a2aq_out")
            kt_ib = dram.tile([D, TC], BF, name="a2ak_in")
            kt_ob = dram.tile([D, TC], BF, name="a2ak_out")
            v_ib = dram.tile([NC, TC, 128], BF, name="a2av_in")
            v_ob = dram.tile([NC, TC, 128], BF, name="a2av_out")

            rqk_fam = wf.tile([128, NRC, D], BF, tag="Rfam", name="rqk_fam")
            nc.sync.dma_start(rqk_fam[:], RQK[:])

            with tc.tile_pool(name="psQ", bufs=1, space="PSUM") as psQ:
                restore_T(u_t, rqk_fam, "qT", psQ, qt_ib,
                          uspec=("wrq", hq_st, selfeat_t))
            nc.gpsimd.collective_compute(
                "AllToAll", ALU.bypass, replica_groups=groups,
                ins=[qt_ib.opt()], outs=[qt_ob.opt()])

            with tc.tile_pool(name="psK", bufs=1, space="PSUM") as psK:
                restore_T(u_t, rqk_fam, "kT", psK, kt_ib,
                          uspec=("wrk", hk_st, selfeat_t))
            nc.gpsimd.collective_compute(
                "AllToAll", ALU.bypass, replica_groups=groups,
                ins=[kt_ib.opt()], outs=[kt_ob.opt()])

            rv_fam = wf.tile([128, NRC, D], BF, tag="Rfam", name="rv_fam")
            nc.sync.dma_start(rv_fam[:], RV[:])
            psT_ctx = tc.tile_pool(name="psATT", bufs=1, space="PSUM")
            psT = psT_ctx.__enter__()
            for qtr in range(4):
                vps = [psT.tile([128, 512], F32, tag=f"rt{k}",
                                name=f"vps{qtr}_{k}") for k in range(2)]
                for c in range(NRC):
                    if qtr == 0:
                        build_u_chunk("wrv", hv_st, u_t[c], c,
                                      selfeat_t, psT, tag="st0", nbufs=1)
                    for k in range(2):
                        i = qtr
                        j = k
                        nc.tensor.matmul(
                            vps[k][:],
                            u_t[c][:, i * 128:(i + 1) * 128],
                            rv_fam[:, c, j * 512:(j + 1) * 512],
                            start=(c == 0), stop=(c == NRC - 1))
                vsb = ch.tile([128, D], BF, tag="vT", bufs=2)
                for j in range(2):
                    nc.scalar.copy(vsb[:, j * 512:(j + 1) * 512],
                                   vps[j][:])
                nc.sync.dma_start(
                    v_ib.rearrange("p t e -> t p e")
                        [qtr * 128:(qtr + 1) * 128],
                    vsb[:].rearrange("q (p e) -> q p e", e=128))
            nc.gpsimd.collective_compute(
                "AllToAll", ALU.bypass, replica_groups=groups,
                ins=[v_ib.opt()], outs=[v_ob.opt()])

            # ---- attention (heads 2c, 2c+1; all tokens) ----------------
            qTh = pp.tile([128, NC, TC], BF, name="qTh", tag="qTh")
            kTh = pp.tile([128, NC, TC], BF, name="kTh", tag="kTh")
            nc.sync.dma_start(qTh[:],
                              qt_ob.rearrange("(r p) t -> p r t", p=128))
            nc.sync.dma_start(kTh[:],
                              kt_ob.rearrange("(r p) t -> p r t", p=128))
            qTf = qTh.rearrange("p r t -> p (r t)")
            kTf = kTh.rearrange("p r t -> p (r t)")

            vi_all = pp.tile([128, 2, T // 128, 65], BF, name="vi_all")
            v_ov = v_ob.rearrange("r (i p) (h e) -> p (r i) h e", p=128, e=64)
            for hp in range(2):
                nc.sync.dma_start(vi_all[:, hp, :, 0:64], v_ov[:, :, hp, :])
            nc.vector.tensor_copy(
                vi_all[:, :, :, 64].rearrange("p a b -> p (a b)"), ones64_t)

            attnT = pp.tile([128, T], BF, name="attnT", tag="attnT")

            NQB = S // 256
            if True:
                st_tiles = [psT.tile([128, 256], F32, tag=f"st{k}",
                                     name=f"st{k}") for k in range(4)]
                ot_tiles = [psT.tile([65, 256], F32, tag=f"rt{k}",
                                     name=f"ot{k}") for k in range(2)]
                op_tiles = [psT.tile([128, 128], BF, tag=f"op{k}",
                                     name=f"op{k}") for k in range(2)]
                sti, oti, opi = [0], [0], [0]

                def next_t(tiles, idx):
                    t = tiles[idx[0] % len(tiles)]
                    idx[0] += 1
                    return t

                LA = 2   # S^T/exp lookahead depth before each PV
                for b in range(B):
                    for qb in range(NQB):
                        q0 = b * S + qb * 256
                        aN0 = ch.tile([128, 128], BF, tag="aN", bufs=4)
                        aN1 = ch.tile([128, 128], BF, tag="aN", bufs=4)
                        aNs = [aN0, aN1]
                        ots_hp = [next_t(ot_tiles, oti) for _ in range(2)]
                        iters = [(hp, kb, kc)
                                 for kb in range(qb + 1)
                                 for kc in range(2)
                                 for hp in range(2)]
                        esq = []

                        def emit_st(hp, kb, kc):
                            k0 = b * S + kb * 256 + kc * 128
                            st = next_t(st_tiles, sti)
                            nc.tensor.matmul(
                                st[:],
                                kTf[hp * 64:(hp + 1) * 64, k0:k0 + 128],
                                qTf[hp * 64:(hp + 1) * 64, q0:q0 + 256],
                                start=True, stop=True)
                            es = ch.tile([128, 256], BF, tag="es", bufs=6)
                            nc.scalar.activation(es[:], st[:], AF.Exp,
                                                 scale=0.125)
                            if kb == qb:
                                nc.vector.tensor_mul(es[:], es[:],
                                                     mask_t[kc])
                            return es

                        def emit_pv(es, hp, kb, kc):
                            k0 = b * S + kb * 256 + kc * 128
                            nc.tensor.matmul(
                                ots_hp[hp][:], vi_all[:, hp, k0 // 128, :],
                                es[:],
                                start=(kb == 0 and kc == 0),
                                stop=(kb == qb and kc == 1))

                        for j, (hp, kb, kc) in enumerate(iters):
                            esq.append((emit_st(hp, kb, kc), hp, kb, kc))
                            if len(esq) > LA:
                                emit_pv(*esq.pop(0))
                        while esq:
                            emit_pv(*esq.pop(0))

                        for hp in range(2):
                            ots = ch.tile([65, 256], BF, tag="ots")
                            nc.scalar.copy(ots[:], ots_hp[hp][:])
                            for qc in range(2):
                                op = next_t(op_tiles, opi)
                                nc.tensor.transpose(
                                    op[:, 0:65],
                                    ots[:, qc * 128:(qc + 1) * 128],
                                    ident_t[0:65, 0:65])
                                oq = ch.tile([128, 65], F32, tag="oq")
                                nc.vector.tensor_copy(oq[:], op[:, 0:65])
                                rz = ch.tile([128, 1], F32, tag="rz")
                                nc.vector.reciprocal(rz[:], oq[:, 64:65])
                                nc.vector.tensor_scalar(
                                    aNs[qc][:, hp * 64:(hp + 1) * 64],
                                    oq[:, 0:64], rz[:], None, ALU.mult)
                        for qc in range(2):
                            op = next_t(op_tiles, opi)
                            nc.tensor.transpose(op[:], aNs[qc][:], ident_t)
                            ti = q0 + qc * 128
                            nc.scalar.copy(attnT[:, ti:ti + 128], op[:])

            psT_ctx.__exit__(None, None, None)
            at_ib = dram.tile([NC, 128, TC], BF, name="a2aa_in")
            at_ob = dram.tile([NC, 128, TC], BF, name="a2aa_out")
            nc.sync.dma_start(at_ib.rearrange("r p t -> p r t"),
                              attnT[:].rearrange("p (r t) -> p r t", t=TC))
            nc.gpsimd.collective_compute(
                "AllToAll", ALU.bypass, replica_groups=groups,
                ins=[at_ib.opt()], outs=[at_ob.opt()])

            atT = pp.tile([128, NC, TC], BF, name="atT", tag="attnT")
            nc.sync.dma_start(atT[:], at_ob.rearrange("r p t -> p r t"))

            # ---- W_O + residual (in-place into x_t) --------------------
            wot_fam = wf.tile([128, DC, D], BF, tag="Rfam", name="wot_fam")
            nc.sync.dma_start(wot_fam[:], WOT[:])
            with tc.tile_pool(name="psWO", bufs=1, space="PSUM") as psW:
                aops = [psW.tile([128, 512], F32, tag=f"ao{k}",
                                 name=f"ao{k}") for k in range(NT * 2)]
                for dc in range(DC):
                    for i in range(NT):
                        for j in range(2):
                            nc.tensor.matmul(
                                aops[i * 2 + j][:],
                                atT[:, dc, i * 128:(i + 1) * 128],
                                wot_fam[:, dc, j * 512:(j + 1) * 512],
                                start=(dc == 0), stop=(dc == DC - 1))
                for i in range(NT):
                    for j in range(2):
                        sl = slice(j * 512, (j + 1) * 512)
                        nc.vector.tensor_add(x_t[i][:, sl], x_t[i][:, sl],
                                             aops[i * 2 + j][:])

            # ---- LN2 + knowledge ---------------------------------------
            z2T = [pp.tile([128, TC], BF, name=f"z2T{dc}", tag=f"zT{dc}")
                   for dc in range(DC)]
            with tc.tile_pool(name="psLN2", bufs=2, space="PSUM") as psLN2:
                layernorm_zT(x_t, z2T, psLN2, "b")

            selkn_t = pp.tile([N, NKC, 128], BF, name="selkn_t",
                              tag="vi_all")
            nc.sync.dma_start(selkn_t[:], selkn[:])
            h2_sb = pp.tile([128, TC], BF, name="h2_sb")
            with tc.tile_pool(name="psKF", bufs=1, space="PSUM") as psK2:
                h2_ps = psK2.tile([128, TC], F32, tag="h2", name="h2_ps")
                pend2 = []

                def flush_pend2():
                    for gs, c in pend2:
                        nc.tensor.matmul(h2_ps[:], ident_t, gs[:],
                                         start=(c == 0),
                                         stop=(c == NKC - 1))
                    pend2.clear()

                for half in range(2):
                    fam = wf.tile([128, DC, 2048], BF, tag="Ffam",
                                  name=f"fkn{half}")
                    nc.sync.dma_start(
                        fam[:], FKN[:, :, half * 2048:(half + 1) * 2048])
                    for cg in range(4):
                        gtp = [psK2.tile([128, TC], F32, tag=f"g{k}",
                                         name=f"g2{half}{cg}_{k}")
                               for k in range(4)]
                        for k in range(4):
                            c = (half * 4 + cg) * 4 + k
                            if with_bv:
                                nc.tensor.matmul(
                                    gtp[k][:],
                                    bvb_t[:, bv_of["kn"] + c * 128:
                                          bv_of["kn"] + (c + 1) * 128],
                                    onesr_t, start=True, stop=False)
                        for dc in range(DC):
                            for k in range(4):
                                ci = cg * 4 + k
                                nc.tensor.matmul(
                                    gtp[k][:],
                                    fam[:, dc, ci * 128:(ci + 1) * 128],
                                    z2T[dc][:],
                                    start=(not with_bv and dc == 0),
                                    stop=(dc == DC - 1))
                            if dc == 0:
                                flush_pend2()
                        for k in range(4):
                            c = (half * 4 + cg) * 4 + k
                            gcp = ch.tile([128, TC], BF, tag="gcp", bufs=4)
                            nc.scalar.copy(gcp[:], gtp[k][:])
                            wb = psK2.tile([128, TC], F32, tag="wb", bufs=2)
                            nc.tensor.matmul(wb[:], selkn_t[:, c, :],
                                             wt_t["wkf"],
                                             start=True, stop=True)
                            gs = ch.tile([128, TC], BF, tag="gs", bufs=8)
                            nc.vector.tensor_mul(gs[:], gcp[:], wb[:])
                            pend2.append((gs, c))
                flush_pend2()
                nc.scalar.copy(h2_sb[:], h2_ps[:])

            u2_b = pp.tile([128, DC, TC], BF, name="u2_b", tag="qTh")
            u2_c = pp.tile([128, DC, TC], BF, name="u2_c", tag="kTh")
            u2_t = [pp.tile([128, TC], BF, name=f"u2_t{c}", tag=f"u{c}")
                    for c in range(NRC)]
            u2_t += [u2_b[:, k, :] for k in range(DC)]
            u2_t += [u2_c[:, k, :] for k in range(DC)]

            with tc.tile_pool(name="psKR", bufs=1, space="PSUM") as psKR:
                for hp in range(2):
                    kps = [psKR.tile([128, 512], F32, tag=f"rt{k}",
                                     name=f"kp{hp}_{k}") for k in range(4)]
                    for half in range(2):
                        fam = wf.tile([128, NRC, D], BF, tag="Rfam",
                                      name=f"rkn{hp}_{half}")
                        nc.sync.dma_start(
                            fam[:],
                            RKN[:, half * NRC:(half + 1) * NRC, :])
                        for cc in range(NRC):
                            c = half * NRC + cc
                            if hp == 0:
                                build_u_chunk("wkr", h2_sb, u2_t[c], c,
                                              selkn_t, psKR)
                            for k in range(4):
                                i = hp * 2 + k // 2
                                j = k % 2
                                nc.tensor.matmul(
                                    kps[k][:],
                                    u2_t[c][:, i * 128:(i + 1) * 128],
                                    fam[:, cc, j * 512:(j + 1) * 512],
                                    start=(c == 0), stop=(c == NKC - 1))
                    for i2 in range(2):
                        i = hp * 2 + i2
                        for j in range(2):
                            sl = slice(j * 512, (j + 1) * 512)
                            nc.vector.tensor_add(
                                x_t[i][:, sl], x_t[i][:, sl],
                                kps[i2 * 2 + j][:])
                        nc.sync.dma_start(y_sh[i * 128:(i + 1) * 128, :],
                                          x_t[i][:])

    split_waits(nc)
    return nc


# ---------------------------------------------------------------------------
# Host side
# ---------------------------------------------------------------------------

_NC_CACHE = {}


def _get_nc(with_bv=False):
    key = ("nc", with_bv)
    if key not in _NC_CACHE:
        _NC_CACHE[key] = build_kernel(with_bv)
    return _NC_CACHE[key]


def _bf16(a):
    import ml_dtypes
    return np.ascontiguousarray(
        np.asarray(a, dtype=np.float32)).astype(ml_dtypes.bfloat16)


def _chunk_major(a, nchunks):
    # [nchunks*128, M] -> [128, nchunks, M]
    M = a.shape[1]
    return np.ascontiguousarray(
        a.reshape(nchunks, 128, M).transpose(1, 0, 2))


def _selfeat():
    m = np.zeros((N, NRC, 128), np.float32)
    for c in range(NRC):
        m[2 * c, c, 0:64] = 1.0
        m[2 * c + 1, c, 64:128] = 1.0
    return m


def _selkn():
    m = np.zeros((N, NKC, 128), np.float32)
    for c in range(NKC):
        m[c, c, :] = 1.0
    return m


def prepare_inputs(x, f_qk, f_v, r_qk, r_v, f_know, r_know, W_O,
                   gamma1, beta1, gamma2, beta2,
                   w_fq, w_fk, w_fv, w_rq, w_rk, w_rv, w_know_f, w_know_r):
    x = np.asarray(x, np.float32).reshape(T, D)
    gamma1 = np.asarray(gamma1, np.float32)
    beta1 = np.asarray(beta1, np.float32)
    gamma2 = np.asarray(gamma2, np.float32)
    beta2 = np.asarray(beta2, np.float32)

    FQKh = (np.asarray(f_qk, np.float32) * gamma1[None, :, None]) \
        .transpose(1, 0, 2).reshape(D, N * R)
    FVh = (np.asarray(f_v, np.float32) * gamma1[None, :, None]) \
        .transpose(1, 0, 2).reshape(D, N * R)
    FKNh = (np.asarray(f_know, np.float32) * gamma2[None, :, None]) \
        .transpose(1, 0, 2).reshape(D, N * RK)
    RQKh = np.asarray(r_qk, np.float32).reshape(N * R, D)
    RVh = np.asarray(r_v, np.float32).reshape(N * R, D)
    RKNh = np.asarray(r_know, np.float32).reshape(N * RK, D)
    WOTh = np.ascontiguousarray(np.asarray(W_O, np.float32).T)

    with_bv = bool(np.any(beta1 != 0) or np.any(beta2 != 0))

    qi = np.arange(256)[None, :]
    ki = np.arange(128)[:, None]
    cblob = np.zeros((128, CB_COLS), np.float32)
    cblob[:, CB_IDENT:CB_IDENT + 128] = np.eye(128)
    cblob[0:64, CB_SEL64:CB_SEL64 + 64] = np.eye(64)
    cblob[64:128, CB_SEL64:CB_SEL64 + 64] = np.eye(64)
    cblob[:, CB_MASK0:CB_MASK0 + 256] = (qi >= ki)
    cblob[:, CB_MASK1:CB_MASK1 + 256] = (qi >= ki + 128)
    cblob[:, CB_ONES:CB_ONES + 64] = 1.0
    oblob = np.ones((1, 128 + TC), np.float32)

    shared = {
        "FQK": _bf16(_chunk_major(FQKh, DC)),
        "FV": _bf16(_chunk_major(FVh, DC)),
        "FKN": _bf16(_chunk_major(FKNh, DC)),
        "RQK": _bf16(_chunk_major(RQKh, NRC)),
        "RV": _bf16(_chunk_major(RVh, NRC)),
        "RKN": _bf16(_chunk_major(RKNh, NKC)),
        "WOT": _bf16(_chunk_major(WOTh, DC)),
        "selfeat": _bf16(_selfeat()),
        "selkn": _bf16(_selkn()),
        "cblob": _bf16(cblob),
        "oblob": _bf16(oblob),
    }
    if with_bv:
        shared["bvb"] = _bf16(np.concatenate(
            [beta1 @ FQKh, beta1 @ FVh, beta2 @ FKNh])[None, :])

    wmap = {"wfq": w_fq, "wfk": w_fk, "wfv": w_fv, "wrq": w_rq,
            "wrk": w_rk, "wrv": w_rv, "wkf": w_know_f, "wkr": w_know_r}
    in_maps = []
    for c in range(NC):
        m = dict(shared)
        m["x_sh"] = np.ascontiguousarray(x[c * TC:(c + 1) * TC])
        wt = np.stack([
            np.asarray(wmap[nm], np.float32).reshape(T, N)
            [c * TC:(c + 1) * TC].T
            for nm in W_ORDER], axis=1)   # [N, 8, TC]
        m["wts"] = _bf16(wt)
        in_maps.append(m)
    return in_maps, with_bv


def assemble_output(results):
    out = np.empty((T, D), np.float32)
    for c in range(NC):
        out[c * TC:(c + 1) * TC] = results[c]["y_sh"]
    return out.reshape(B, S, D)


def kernel(**inputs):
    in_maps, with_bv = prepare_inputs(**inputs)
    nc = _get_nc(with_bv)
    res = run_bass_kernel_spmd(nc, in_maps, list(range(NC)))
    return assemble_output(res.results)


if __name__ == "__main__":
    build_kernel()
    print("kernel built OK")


# revision 23
# speedup vs baseline: 20.6505x; 18.4461x over previous
"""Trainium2 Bass kernel for nn_DAWN_88124138979393 (moe_routing).

Sharding (8 NeuronCores, SPMD — identical instruction stream per core):
  - Token-parallel LN1 / feature / restore / W_O / LN2 / knowledge stages:
    core c owns tokens [c*512, (c+1)*512) of the flattened [4096, 1024]
    residual stream.
  - Head-parallel causal attention: Q^T/K^T/V are exchanged with three
    AllToAlls so core c holds heads {2c, 2c+1} for ALL tokens; the
    normalized attention output returns via a fourth AllToAll.  This keeps
    the causal loop structure identical on every core.
  - All matmuls run in bf16 (PSUM accumulation fp32).  LN statistics,
    softmax normalization and the residual adds stay fp32.
  - gamma1/gamma2 fold into the feature weights on the host; nonzero
    beta1/beta2 enter as a rank-1 correction (beta @ F) via K=1 matmuls
    (emitted only when some beta is nonzero — they are zeros here).
  - Softmax skips max-subtraction (scores are O(1e-3) here; exp cannot
    overflow) and defers normalization: PV accumulates unnormalized
    exp-weights, a ones-column appended to V yields Z in the same matmul,
    and 1/Z is applied per-token after transposing back to token-major.
  - Weights ship in chunk-major host layouts so each weight matrix arrives
    in O(1) large DMAs (per-DMA fixed overhead dominates otherwise).
"""
import sys

sys.path.insert(0, '/opt/trn_rl_repo')

import numpy as np

import bass_rust as _bass_rust
import concourse.bass as bass
import concourse.mybir as mybir
from concourse import tile
from concourse.bass_utils import run_bass_kernel_spmd

dt = mybir.dt
AF = mybir.ActivationFunctionType
ALU = mybir.AluOpType
BF = dt.bfloat16
F32 = dt.float32

B, S, D, H, R, N, RK = 2, 2048, 1024, 16, 64, 32, 128
DH = D // H          # 64
T = B * S            # 4096
NC = 8               # cores
TC = T // NC         # 512 tokens per core
NT = TC // 128       # 4 token tiles per core
NRC = (N * R) // 128   # 16 chunks in the qk/v pools (2 neurons per chunk)
NKC = (N * RK) // 128  # 32 chunks in the knowledge pool (1 neuron per chunk)
DC = D // 128        # 8 d-chunks
EPS = 1e-5

MAX_WAITS_PER_INST = 1

# const blob column offsets (bf16 [128, 768])
CB_IDENT = 0      # [128, 128] identity
CB_SEL64 = 128    # [128, 64]  I64 stacked twice
CB_MASK0 = 192    # [128, 256] causal mask, diag chunk 0
CB_MASK1 = 448    # [128, 256] causal mask, diag chunk 1
CB_ONES = 704     # [128, 64]  ones (V-interleave Z column fill)
CB_COLS = 768

W_ORDER = ("wfq", "wfk", "wfv", "wrq", "wrk", "wrv", "wkf", "wkr")


# ---------------------------------------------------------------------------
# Tile tail-drain patch: walrus in this container rejects instructions that
# carry more than one sync-wait command.  Split the kernel-tail drain into
# one drain per proc, and post-split every instruction's waits onto NOPs.
# ---------------------------------------------------------------------------

def _split_drain_and_barrier(self, tick_clock, wait_clock):
    gc = tick_clock.global_clock
    ticks = list(gc)
    procs = [i for i, t in enumerate(ticks) if t > 0]
    for g in range(0, max(len(procs), 1), MAX_WAITS_PER_INST):
        group = procs[g:g + MAX_WAITS_PER_INST]
        sub = _bass_rust.VectorClock()
        for i in group:
            sub.require_at_least(i, ticks[i])
        drain_inst = self.nc.sync.drain()
        wait_clock.add_sem_waits(
            drain_inst.ins, _bass_rust.ScopedClock({None: sub})
        )
    self.nc.all_engine_barrier()
    assert self.sems is not None
    popped = self.nc._tile_sem_poison_stack.pop()
    assert popped is self._sem_poison
    self.nc.clear_and_free_semaphores(list(self.sems.allocated().values()))
    self.nc.all_engine_barrier()


tile.TileContext._drain_and_barrier = _split_drain_and_barrier


def split_waits(nc, max_waits=MAX_WAITS_PER_INST):
    counter = [0]
    for f in nc.m.functions:
        for blk in f.blocks:
            i = 0
            while i < len(blk.instructions):
                inst = blk.instructions[i]
                si = inst.sync_info
                if si is not None and len(si.on_wait) > max_waits:
                    waits = list(si.on_wait)
                    si.on_wait = waits[:max_waits]
                    extra = waits[max_waits:]
                    for g in range(0, len(extra), max_waits):
                        nop = _bass_rust.InstNoOp(
                            name=f"WSPLIT-{counter[0]}", ins=[], outs=[])
                        counter[0] += 1
                        nop.engine = inst.engine
                        nop.sync_info = mybir.SyncInfo(
                            on_wait=extra[g:g + max_waits], on_update=[])
                        nc.register_instruction(nop, overwrite=True)
                        blk.instructions.insert(i, nop)
                        i += 1
                i += 1
    return nc


# ---------------------------------------------------------------------------
# Kernel builder
# ---------------------------------------------------------------------------

def build_kernel(with_bv=False):
    nc = bass.Bass()

    x_sh = nc.declare_dram_parameter("x_sh", [TC, D], F32, isOutput=False)
    # chunk-major weight layouts (see prepare_inputs)
    FQK = nc.declare_dram_parameter("FQK", [128, DC, N * R], BF, isOutput=False)
    FV = nc.declare_dram_parameter("FV", [128, DC, N * R], BF, isOutput=False)
    FKN = nc.declare_dram_parameter("FKN", [128, DC, N * RK], BF, isOutput=False)
    RQK = nc.declare_dram_parameter("RQK", [128, NRC, D], BF, isOutput=False)
    RV = nc.declare_dram_parameter("RV", [128, NRC, D], BF, isOutput=False)
    RKN = nc.declare_dram_parameter("RKN", [128, NKC, D], BF, isOutput=False)
    WOT = nc.declare_dram_parameter("WOT", [128, DC, D], BF, isOutput=False)
    wts = nc.declare_dram_parameter("wts", [N, len(W_ORDER), TC], BF,
                                    isOutput=False)
    selfeat = nc.declare_dram_parameter("selfeat", [N, NRC, 128], BF,
                                        isOutput=False)
    selkn = nc.declare_dram_parameter("selkn", [N, NKC, 128], BF,
                                      isOutput=False)
    cblob = nc.declare_dram_parameter("cblob", [128, CB_COLS], BF,
                                      isOutput=False)
    oblob = nc.declare_dram_parameter("oblob", [1, 128 + TC], BF,
                                      isOutput=False)
    if with_bv:
        bvb = nc.declare_dram_parameter(
            "bvb", [1, 2 * N * R + N * RK], BF, isOutput=False)

    y_sh = nc.declare_dram_parameter("y_sh", [TC, D], F32, isOutput=True)

    groups = [list(range(NC))]

    with tile.TileContext(nc) as tc:
        with (
            tc.tile_pool(name="const", bufs=1) as cpool,
            tc.tile_pool(name="persist", bufs=1) as pp,
            tc.tile_pool(name="wfam", bufs=1) as wf,
            tc.tile_pool(name="chunk", bufs=3) as ch,
            tc.tile_pool(name="dram", bufs=1, space="DRAM") as dram,
        ):
            # ---- constants (a handful of blob DMAs) --------------------
            cb = cpool.tile([128, CB_COLS], BF, name="cb")
            nc.sync.dma_start(cb[:], cblob[:])
            ident_t = cb[:, CB_IDENT:CB_IDENT + 128]
            sel64_t = cb[:, CB_SEL64:CB_SEL64 + 64]
            mask_t = [cb[:, CB_MASK0:CB_MASK0 + 256],
                      cb[:, CB_MASK1:CB_MASK1 + 256]]
            ones64_t = cb[:, CB_ONES:CB_ONES + 64]
            ob = cpool.tile([1, 128 + TC], BF, name="ob")
            nc.sync.dma_start(ob[:], oblob[:])
            onesr_t = ob[:, 128:128 + TC]
            eps_t = cpool.tile([128, 1], F32, name="eps_t")
            nc.gpsimd.memset(eps_t[:], EPS)
            wt_all = cpool.tile([N, len(W_ORDER), TC], BF, name="wt_all")
            nc.sync.dma_start(wt_all[:], wts[:])
            wt_t = {nm: wt_all[:, wi, :] for wi, nm in enumerate(W_ORDER)}
            selfeat_t = cpool.tile([N, NRC, 128], BF, name="selfeat_t")
            nc.sync.dma_start(selfeat_t[:], selfeat[:])

            if with_bv:
                bvb_t = cpool.tile([1, 2 * N * R + N * RK], BF, name="bvb_t")
                nc.sync.dma_start(bvb_t[:], bvb[:])
                bv_of = {"qk": 0, "v": N * R, "kn": 2 * N * R}

            x_t = [pp.tile([128, D], F32, name=f"x_t{i}") for i in range(NT)]
            for i in range(NT):
                nc.sync.dma_start(x_t[i][:], x_sh[i * 128:(i + 1) * 128, :])

            def layernorm_zT(xtiles, zT_tiles, pool, tag):
                for i in range(NT):
                    mu = ch.tile([128, 1], F32, tag="mu")
                    nc.vector.reduce_sum(mu[:], xtiles[i][:],
                                         axis=mybir.AxisListType.X)
                    nc.vector.tensor_scalar_mul(mu[:], mu[:], 1.0 / D)
                    cen = ch.tile([128, D], F32, tag="cen", bufs=2)
                    nc.vector.tensor_scalar(cen[:], xtiles[i][:], mu[:],
                                            None, ALU.subtract)
                    sq = ch.tile([128, D], F32, tag="cen", bufs=2)
                    var = ch.tile([128, 1], F32, tag="var")
                    nc.scalar.activation(sq[:], cen[:], AF.Square,
                                         accum_out=var[:])
                    sd = ch.tile([128, 1], F32, tag="sd")
                    nc.scalar.activation(sd[:], var[:], AF.Sqrt,
                                         scale=1.0 / D, bias=eps_t[:])
                    rsig = ch.tile([128, 1], F32, tag="rsig")
                    nc.vector.reciprocal(rsig[:], sd[:])
                    zb = ch.tile([128, D], BF, tag="zb")
                    nc.vector.tensor_scalar(zb[:], cen[:], rsig[:],
                                            None, ALU.mult)
                    for dc in range(DC):
                        ztp = pool.tile([128, 128], BF, tag=f"ztp{tag}")
                        nc.tensor.transpose(
                            ztp[:], zb[:, dc * 128:(dc + 1) * 128], ident_t)
                        nc.scalar.copy(
                            zT_tiles[dc][:, i * 128:(i + 1) * 128], ztp[:])

            zT = [pp.tile([128, TC], BF, name=f"zT{dc}", tag=f"zT{dc}")
                  for dc in range(DC)]
            with tc.tile_pool(name="psLN", bufs=2, space="PSUM") as psLN:
                layernorm_zT(x_t, zT, psLN, "a")

            # ---- feature pools -----------------------------------------
            def feature_half(Fdram, half, bvkey, wh_list, nchunks, psF, hs,
                             tag, pending):
                # pending: list of deferred (h_ps, gs, c) SEL matmuls from
                # the previous chunk group — emitted during this group's
                # gt accumulation so the PE never stalls on the DVE muls.
                fam = wf.tile([128, DC, 1024], BF, tag="Ffam",
                              name=f"F{tag}{half}")
                nc.sync.dma_start(
                    fam[:], Fdram[:, :, half * 1024:(half + 1) * 1024])

                def flush_pending():
                    for h_ps, gs, c in pending:
                        nc.tensor.matmul(h_ps[:], sel64_t, gs[:],
                                         start=(c == 0),
                                         stop=(c == nchunks - 1))
                    pending.clear()

                for cg in range(2):        # 2 groups of 4 chunks per half
                    gtp = [psF.tile([128, TC], F32, tag=f"g{k}",
                                    name=f"g{tag}{half}{cg}_{k}")
                           for k in range(4)]
                    for k in range(4):
                        c = (half * 2 + cg) * 4 + k
                        if with_bv:
                            nc.tensor.matmul(
                                gtp[k][:],
                                bvb_t[:, bv_of[bvkey] + c * 128:
                                      bv_of[bvkey] + (c + 1) * 128],
                                onesr_t, start=True, stop=False)
                    for dc in range(DC):
                        for k in range(4):
                            ci = cg * 4 + k
                            nc.tensor.matmul(
                                gtp[k][:],
                                fam[:, dc, ci * 128:(ci + 1) * 128],
                                zT[dc][:],
                                start=(not with_bv and dc == 0),
                                stop=(dc == DC - 1))
                        if dc == 0:
                            flush_pending()
                    for k in range(4):
                        c = (half * 2 + cg) * 4 + k
                        gcp = ch.tile([128, TC], BF, tag="gcp", bufs=4)
                        nc.scalar.copy(gcp[:], gtp[k][:])
                        for wi, wname in enumerate(wh_list):
                            wb = psF.tile([128, TC], F32, tag="wb", bufs=2)
                            nc.tensor.matmul(wb[:], selfeat_t[:, c, :],
                                             wt_t[wname],
                                             start=True, stop=True)
                            gs = ch.tile([128, TC], BF, tag="gs", bufs=8)
                            nc.vector.tensor_mul(gs[:], gcp[:], wb[:])
                            pending.append((hs[wi], gs, c))

            def hstack(h_ps, name):
                t = pp.tile([128, TC], BF, name=name)
                nc.scalar.copy(t[0:64, :], h_ps[:])
                nc.scalar.copy(t[64:128, :], h_ps[:])
                return t

            with tc.tile_pool(name="psF", bufs=1, space="PSUM") as psF:
                hq_ps = psF.tile([64, TC], F32, tag="hq", name="hq_ps")
                hk_ps = psF.tile([64, TC], F32, tag="hk", name="hk_ps")
                pend = []
                for half in range(2):
                    feature_half(FQK, half, "qk", ["wfq", "wfk"], NRC, psF,
                                 [hq_ps, hk_ps], "qk", pend)
                for h_ps, gs, c in pend:
                    nc.tensor.matmul(h_ps[:], sel64_t, gs[:],
                                     start=(c == 0), stop=(c == NRC - 1))
                pend.clear()
                hq_st = hstack(hq_ps, "hq_st")
                hk_st = hstack(hk_ps, "hk_st")
                hv_ps = psF.tile([64, TC], F32, tag="hq", name="hv_ps")
                for half in range(2):
                    feature_half(FV, half, "v", ["wfv"], NRC, psF,
                                 [hv_ps], "v", pend)
                for h_ps, gs, c in pend:
                    nc.tensor.matmul(h_ps[:], sel64_t, gs[:],
                                     start=(c == 0), stop=(c == NRC - 1))
                pend.clear()
                hv_st = hstack(hv_ps, "hv_st")

            # ---- u chunks + restores -----------------------------------
            u_t = [pp.tile([128, TC], BF, name=f"u_t{c}", tag=f"u{c}")
                   for c in range(NRC)]

            def build_u_chunk(wname, h_st, ut, c, seln, psU):
                wb = psU.tile([128, TC], F32, tag="uwb", bufs=2)
                nc.tensor.matmul(wb[:], seln[:, c, :], wt_t[wname],
                                 start=True, stop=True)
                nc.vector.tensor_mul(ut[:], h_st[:], wb[:])

            def restore_T(u_tiles, Rfam, out_name, psR, ib,
                          uspec=None):
                # two passes of 4 psum banks; pass 0 builds u on the fly
                for hp in range(2):
                    ps = [psR.tile([128, TC], F32, tag=f"rt{k}",
                                   name=f"{out_name}ps{hp}_{k}")
                          for k in range(4)]
                    for c in range(NRC):
                        if hp == 0 and uspec is not None:
                            build_u_chunk(uspec[0], uspec[1], u_tiles[c],
                                          c, uspec[2], psR)
                        for k in range(4):
                            dc = hp * 4 + k
                            nc.tensor.matmul(
                                ps[k][:],
                                Rfam[:, c, dc * 128:(dc + 1) * 128],
                                u_tiles[c][:], start=(c == 0),
                                stop=(c == NRC - 1))
                    for k in range(4):
                        dc = hp * 4 + k
                        ot = ch.tile([128, TC], BF, tag="rT")
                        nc.scalar.copy(ot[:], ps[k][:])
                        nc.sync.dma_start(ib[dc * 128:(dc + 1) * 128, :],
                                          ot[:])

            qt_ib = dram.tile([D, TC], BF, name="a2aq_in")
            qt_ob = dram.tile([D, TC], BF, name="a2aq_out")
            kt_ib = dram.tile([D, TC], BF, name="a2ak_in")
            kt_ob = dram.tile([D, TC], BF, name="a2ak_out")
            v_ib = dram.tile([NC, TC, 128], BF, name="a2av_in")
            v_ob = dram.tile([NC, TC, 128], BF, name="a2av_out")

            rqk_fam = wf.tile([128, NRC, D], BF, tag="Rfam", name="rqk_fam")
            nc.sync.dma_start(rqk_fam[:], RQK[:])

            with tc.tile_pool(name="psQ", bufs=1, space="PSUM") as psQ:
                restore_T(u_t, rqk_fam, "qT", psQ, qt_ib,
                          uspec=("wrq", hq_st, selfeat_t))
            nc.gpsimd.collective_compute(
                "AllToAll", ALU.bypass, replica_groups=groups,
                ins=[qt_ib.opt()], outs=[qt_ob.opt()])

            with tc.tile_pool(name="psK", bufs=1, space="PSUM") as psK:
                restore_T(u_t, rqk_fam, "kT", psK, kt_ib,
                          uspec=("wrk", hk_st, selfeat_t))
            nc.gpsimd.collective_compute(
                "AllToAll", ALU.bypass, replica_groups=groups,
                ins=[kt_ib.opt()], outs=[kt_ob.opt()])

            rv_fam = wf.tile([128, NRC, D], BF, tag="Rfam", name="rv_fam")
            nc.sync.dma_start(rv_fam[:], RV[:])
            with tc.tile_pool(name="psV", bufs=1, space="PSUM") as psV:
                for half in range(2):
                    vps = [psV.tile([128, 512], F32, tag=f"rt{k}",
                                    name=f"vps{half}_{k}") for k in range(4)]
                    for c in range(NRC):
                        if half == 0:
                            build_u_chunk("wrv", hv_st, u_t[c], c,
                                          selfeat_t, psV)
                        for k in range(4):
                            i = half * 2 + k // 2
                            j = k % 2
                            nc.tensor.matmul(
                                vps[k][:],
                                u_t[c][:, i * 128:(i + 1) * 128],
                                rv_fam[:, c, j * 512:(j + 1) * 512],
                                start=(c == 0), stop=(c == NRC - 1))
                    for i2 in range(2):
                        i = half * 2 + i2
                        vsb = ch.tile([128, D], BF, tag="vT", bufs=2)
                        for j in range(2):
                            nc.scalar.copy(vsb[:, j * 512:(j + 1) * 512],
                                           vps[i2 * 2 + j][:])
                        nc.sync.dma_start(
                            v_ib.rearrange("p t e -> t p e")
                                [i * 128:(i + 1) * 128],
                            vsb[:].rearrange("q (p e) -> q p e", e=128))
            nc.gpsimd.collective_compute(
                "AllToAll", ALU.bypass, replica_groups=groups,
                ins=[v_ib.opt()], outs=[v_ob.opt()])

            # ---- attention (heads 2c, 2c+1; all tokens) ----------------
            qTh = pp.tile([128, NC, TC], BF, name="qTh", tag="qTh")
            kTh = pp.tile([128, NC, TC], BF, name="kTh", tag="kTh")
            nc.sync.dma_start(qTh[:],
                              qt_ob.rearrange("(r p) t -> p r t", p=128))
            nc.sync.dma_start(kTh[:],
                              kt_ob.rearrange("(r p) t -> p r t", p=128))
            qTf = qTh.rearrange("p r t -> p (r t)")
            kTf = kTh.rearrange("p r t -> p (r t)")

            vi_all = pp.tile([128, 2, T // 128, 65], BF, name="vi_all")
            v_ov = v_ob.rearrange("r (i p) (h e) -> p (r i) h e", p=128, e=64)
            for hp in range(2):
                nc.sync.dma_start(vi_all[:, hp, :, 0:64], v_ov[:, :, hp, :])
            nc.vector.tensor_copy(
                vi_all[:, :, :, 64].rearrange("p a b -> p (a b)"), ones64_t)

            attnT = pp.tile([128, T], BF, name="attnT", tag="attnT")

            NQB = S // 256
            with tc.tile_pool(name="psATT", bufs=1, space="PSUM") as psT:
                st_tiles = [psT.tile([128, 256], F32, tag=f"st{k}",
                                     name=f"st{k}") for k in range(4)]
                ot_tiles = [psT.tile([65, 256], F32, tag=f"ot{k}",
                                     name=f"ot{k}") for k in range(2)]
                op_tiles = [psT.tile([128, 128], BF, tag=f"op{k}",
                                     name=f"op{k}") for k in range(2)]
                sti, oti, opi = [0], [0], [0]

                def next_t(tiles, idx):
                    t = tiles[idx[0] % len(tiles)]
                    idx[0] += 1
                    return t

                LA = 2   # S^T/exp lookahead depth before each PV
                for b in range(B):
                    for qb in range(NQB):
                        q0 = b * S + qb * 256
                        aN0 = ch.tile([128, 128], BF, tag="aN", bufs=4)
                        aN1 = ch.tile([128, 128], BF, tag="aN", bufs=4)
                        aNs = [aN0, aN1]
                        ots_hp = [next_t(ot_tiles, oti) for _ in range(2)]
                        iters = [(hp, kb, kc)
                                 for kb in range(qb + 1)
                                 for kc in range(2)
                                 for hp in range(2)]
                        esq = []

                        def emit_st(hp, kb, kc):
                            k0 = b * S + kb * 256 + kc * 128
                            st = next_t(st_tiles, sti)
                            nc.tensor.matmul(
                                st[:],
                                kTf[hp * 64:(hp + 1) * 64, k0:k0 + 128],
                                qTf[hp * 64:(hp + 1) * 64, q0:q0 + 256],
                                start=True, stop=True)
                            es = ch.tile([128, 256], BF, tag="es", bufs=6)
                            nc.scalar.activation(es[:], st[:], AF.Exp,
                                                 scale=0.125)
                            if kb == qb:
                                nc.vector.tensor_mul(es[:], es[:],
                                                     mask_t[kc])
                            return es

                        def emit_pv(es, hp, kb, kc):
                            k0 = b * S + kb * 256 + kc * 128
                            nc.tensor.matmul(
                                ots_hp[hp][:], vi_all[:, hp, k0 // 128, :],
                                es[:],
                                start=(kb == 0 and kc == 0),
                                stop=(kb == qb and kc == 1))

                        for j, (hp, kb, kc) in enumerate(iters):
                            esq.append((emit_st(hp, kb, kc), hp, kb, kc))
                            if len(esq) > LA:
                                emit_pv(*esq.pop(0))
                        while esq:
                            emit_pv(*esq.pop(0))

                        for hp in range(2):
                            ots = ch.tile([65, 256], BF, tag="ots")
                            nc.scalar.copy(ots[:], ots_hp[hp][:])
                            for qc in range(2):
                                op = next_t(op_tiles, opi)
                                nc.tensor.transpose(
                                    op[:, 0:65],
                                    ots[:, qc * 128:(qc + 1) * 128],
                                    ident_t[0:65, 0:65])
                                oq = ch.tile([128, 65], F32, tag="oq")
                                nc.vector.tensor_copy(oq[:], op[:, 0:65])
                                rz = ch.tile([128, 1], F32, tag="rz")
                                nc.vector.reciprocal(rz[:], oq[:, 64:65])
                                nc.vector.tensor_scalar(
                                    aNs[qc][:, hp * 64:(hp + 1) * 64],
                                    oq[:, 0:64], rz[:], None, ALU.mult)
                        for qc in range(2):
                            op = next_t(op_tiles, opi)
                            nc.tensor.transpose(op[:], aNs[qc][:], ident_t)
                            ti = q0 + qc * 128
                            nc.scalar.copy(attnT[:, ti:ti + 128], op[:])

            at_ib = dram.tile([NC, 128, TC], BF, name="a2aa_in")
            at_ob = dram.tile([NC, 128, TC], BF, name="a2aa_out")
            nc.sync.dma_start(at_ib.rearrange("r p t -> p r t"),
                              attnT[:].rearrange("p (r t) -> p r t", t=TC))
            nc.gpsimd.collective_compute(
                "AllToAll", ALU.bypass, replica_groups=groups,
                ins=[at_ib.opt()], outs=[at_ob.opt()])

            atT = pp.tile([128, NC, TC], BF, name="atT", tag="attnT")
            nc.sync.dma_start(atT[:], at_ob.rearrange("r p t -> p r t"))

            # ---- W_O + residual (in-place into x_t) --------------------
            wot_fam = wf.tile([128, DC, D], BF, tag="Rfam", name="wot_fam")
            nc.sync.dma_start(wot_fam[:], WOT[:])
            with tc.tile_pool(name="psWO", bufs=1, space="PSUM") as psW:
                aops = [psW.tile([128, 512], F32, tag=f"ao{k}",
                                 name=f"ao{k}") for k in range(NT * 2)]
                for dc in range(DC):
                    for i in range(NT):
                        for j in range(2):
                            nc.tensor.matmul(
                                aops[i * 2 + j][:],
                                atT[:, dc, i * 128:(i + 1) * 128],
                                wot_fam[:, dc, j * 512:(j + 1) * 512],
                                start=(dc == 0), stop=(dc == DC - 1))
                for i in range(NT):
                    for j in range(2):
                        sl = slice(j * 512, (j + 1) * 512)
                        nc.vector.tensor_add(x_t[i][:, sl], x_t[i][:, sl],
                                             aops[i * 2 + j][:])

            # ---- LN2 + knowledge ---------------------------------------
            z2T = [pp.tile([128, TC], BF, name=f"z2T{dc}", tag=f"zT{dc}")
                   for dc in range(DC)]
            with tc.tile_pool(name="psLN2", bufs=2, space="PSUM") as psLN2:
                layernorm_zT(x_t, z2T, psLN2, "b")

            selkn_t = pp.tile([N, NKC, 128], BF, name="selkn_t",
                              tag="vi_all")
            nc.sync.dma_start(selkn_t[:], selkn[:])
            h2_sb = pp.tile([128, TC], BF, name="h2_sb")
            with tc.tile_pool(name="psKF", bufs=1, space="PSUM") as psK2:
                h2_ps = psK2.tile([128, TC], F32, tag="h2", name="h2_ps")
                pend2 = []

                def flush_pend2():
                    for gs, c in pend2:
                        nc.tensor.matmul(h2_ps[:], ident_t, gs[:],
                                         start=(c == 0),
                                         stop=(c == NKC - 1))
                    pend2.clear()

                for half in range(2):
                    fam = wf.tile([128, DC, 2048], BF, tag="Ffam",
                                  name=f"fkn{half}")
                    nc.sync.dma_start(
                        fam[:], FKN[:, :, half * 2048:(half + 1) * 2048])
                    for cg in range(4):
                        gtp = [psK2.tile([128, TC], F32, tag=f"g{k}",
                                         name=f"g2{half}{cg}_{k}")
                               for k in range(4)]
                        for k in range(4):
                            c = (half * 4 + cg) * 4 + k
                            if with_bv:
                                nc.tensor.matmul(
                                    gtp[k][:],
                                    bvb_t[:, bv_of["kn"] + c * 128:
                                          bv_of["kn"] + (c + 1) * 128],
                                    onesr_t, start=True, stop=False)
                        for dc in range(DC):
                            for k in range(4):
                                ci = cg * 4 + k
                                nc.tensor.matmul(
                                    gtp[k][:],
                                    fam[:, dc, ci * 128:(ci + 1) * 128],
                                    z2T[dc][:],
                                    start=(not with_bv and dc == 0),
                                    stop=(dc == DC - 1))
                            if dc == 0:
                                flush_pend2()
                        for k in range(4):
                            c = (half * 4 + cg) * 4 + k
                            gcp = ch.tile([128, TC], BF, tag="gcp", bufs=4)
                            nc.scalar.copy(gcp[:], gtp[k][:])
                            wb = psK2.tile([128, TC], F32, tag="wb", bufs=2)
                            nc.tensor.matmul(wb[:], selkn_t[:, c, :],
                                             wt_t["wkf"],
                                             start=True, stop=True)
                            gs = ch.tile([128, TC], BF, tag="gs", bufs=8)
                            nc.vector.tensor_mul(gs[:], gcp[:], wb[:])
                            pend2.append((gs, c))
                flush_pend2()
                nc.scalar.copy(h2_sb[:], h2_ps[:])

            u2_b = pp.tile([128, DC, TC], BF, name="u2_b", tag="qTh")
            u2_c = pp.tile([128, DC, TC], BF, name="u2_c", tag="kTh")
            u2_t = [pp.tile([128, TC], BF, name=f"u2_t{c}", tag=f"u{c}")
                    for c in range(NRC)]
            u2_t += [u2_b[:, k, :] for k in range(DC)]
            u2_t += [u2_c[:, k, :] for k in range(DC)]

            with tc.tile_pool(name="psKR", bufs=1, space="PSUM") as psKR:
                for hp in range(2):
                    kps = [psKR.tile([128, 512], F32, tag=f"rt{k}",
                                     name=f"kp{hp}_{k}") for k in range(4)]
                    for half in range(2):
                        fam = wf.tile([128, NRC, D], BF, tag="Rfam",
                                      name=f"rkn{hp}_{half}")
                        nc.sync.dma_start(
                            fam[:],
                            RKN[:, half * NRC:(half + 1) * NRC, :])
                        for cc in range(NRC):
                            c = half * NRC + cc
                            if hp == 0:
                                build_u_chunk("wkr", h2_sb, u2_t[c], c,
                                              selkn_t, psKR)
                            for k in range(4):
                                i = hp * 2 + k // 2
                                j = k % 2
                                nc.tensor.matmul(
                                    kps[k][:],
                                    u2_t[c][:, i * 128:(i + 1) * 128],
                                    fam[:, cc, j * 512:(j + 1) * 512],
                                    start=(c == 0), stop=(c == NKC - 1))
                    for i2 in range(2):
                        i = hp * 2 + i2
                        for j in range(2):
                            sl = slice(j * 512, (j + 1) * 512)
                            nc.vector.tensor_add(
                                x_t[i][:, sl], x_t[i][:, sl],
                                kps[i2 * 2 + j][:])
                        nc.sync.dma_start(y_sh[i * 128:(i + 1) * 128, :],
                                          x_t[i][:])

    split_waits(nc)
    return nc


# ---------------------------------------------------------------------------
# Host side
# ---------------------------------------------------------------------------

_NC_CACHE = {}


def _get_nc(with_bv=False):
    key = ("nc", with_bv)
    if key not in _NC_CACHE:
        _NC_CACHE[key] = build_kernel(with_bv)
    return _NC_CACHE[key]


def _bf16(a):
    import ml_dtypes
    return np.ascontiguousarray(
        np.asarray(a, dtype=np.float32)).astype(ml_dtypes.bfloat16)


def _chunk_major(a, nchunks):
    # [nchunks*128, M] -> [128, nchunks, M]
    M = a.shape[1]
    return np.ascontiguousarray(
        a.reshape(nchunks, 128, M).transpose(1, 0, 2))


def _selfeat():
    m = np.zeros((N, NRC, 128), np.float32)
    for c in range(NRC):
        m[2 * c, c, 0:64] = 1.0
        m[2 * c + 1, c, 64:128] = 1.0
    return m


def _selkn():
    m = np.zeros((N, NKC, 128), np.float32)
    for c in range(NKC):
        m[c, c, :] = 1.0
    return m


def prepare_inputs(x, f_qk, f_v, r_qk, r_v, f_know, r_know, W_O,
                   gamma1, beta1, gamma2, beta2,
                   w_fq, w_fk, w_fv, w_rq, w_rk, w_rv, w_know_f, w_know_r):
    x = np.asarray(x, np.float32).reshape(T, D)
    gamma1 = np.asarray(gamma1, np.float32)
    beta1 = np.asarray(beta1, np.float32)
    gamma2 = np.asarray(gamma2, np.float32)
    beta2 = np.asarray(beta2, np.float32)

    FQKh = (np.asarray(f_qk, np.float32) * gamma1[None, :, None]) \
        .transpose(1, 0, 2).reshape(D, N * R)
    FVh = (np.asarray(f_v, np.float32) * gamma1[None, :, None]) \
        .transpose(1, 0, 2).reshape(D, N * R)
    FKNh = (np.asarray(f_know, np.float32) * gamma2[None, :, None]) \
        .transpose(1, 0, 2).reshape(D, N * RK)
    RQKh = np.asarray(r_qk, np.float32).reshape(N * R, D)
    RVh = np.asarray(r_v, np.float32).reshape(N * R, D)
    RKNh = np.asarray(r_know, np.float32).reshape(N * RK, D)
    WOTh = np.ascontiguousarray(np.asarray(W_O, np.float32).T)

    with_bv = bool(np.any(beta1 != 0) or np.any(beta2 != 0))

    qi = np.arange(256)[None, :]
    ki = np.arange(128)[:, None]
    cblob = np.zeros((128, CB_COLS), np.float32)
    cblob[:, CB_IDENT:CB_IDENT + 128] = np.eye(128)
    cblob[0:64, CB_SEL64:CB_SEL64 + 64] = np.eye(64)
    cblob[64:128, CB_SEL64:CB_SEL64 + 64] = np.eye(64)
    cblob[:, CB_MASK0:CB_MASK0 + 256] = (qi >= ki)
    cblob[:, CB_MASK1:CB_MASK1 + 256] = (qi >= ki + 128)
    cblob[:, CB_ONES:CB_ONES + 64] = 1.0
    oblob = np.ones((1, 128 + TC), np.float32)

    shared = {
        "FQK": _bf16(_chunk_major(FQKh, DC)),
        "FV": _bf16(_chunk_major(FVh, DC)),
        "FKN": _bf16(_chunk_major(FKNh, DC)),
        "RQK": _bf16(_chunk_major(RQKh, NRC)),
        "RV": _bf16(_chunk_major(RVh, NRC)),
        "RKN": _bf16(_chunk_major(RKNh, NKC)),
        "WOT": _bf16(_chunk_major(WOTh, DC)),
        "selfeat": _bf16(_selfeat()),
        "selkn": _bf16(_selkn()),
        "cblob": _bf16(cblob),
        "oblob": _bf16(oblob),
    }
    if with_bv:
        shared["bvb"] = _bf16(np.concatenate(
            [beta1 @ FQKh, beta1 @ FVh, beta2 @ FKNh])[None, :])

    wmap = {"wfq": w_fq, "wfk": w_fk, "wfv": w_fv, "wrq": w_rq,
            "wrk": w_rk, "wrv": w_rv, "wkf": w_know_f, "wkr": w_know_r}
    in_maps = []
    for c in range(NC):
        m = dict(shared)
        m["x_sh"] = np.ascontiguousarray(x[c * TC:(c + 1) * TC])
        wt = np.stack([
            np.asarray(wmap[nm], np.float32).reshape(T, N)
            [c * TC:(c + 1) * TC].T
            for nm in W_ORDER], axis=1)   # [N, 8, TC]
        m["wts"] = _bf16(wt)
        in_maps.append(m)
    return in_maps, with_bv


def assemble_output(results):
    out = np.empty((T, D), np.float32)
    for c in range(NC):
        out[c * TC:(c + 1) * TC] = results[c]["y_sh"]
    return out.reshape(B, S, D)


def kernel(**inputs):
    in_maps, with_bv = prepare_inputs(**inputs)
    nc = _get_nc(with_bv)
    res = run_bass_kernel_spmd(nc, in_maps, list(range(NC)))
    return assemble_output(res.results)


if __name__ == "__main__":
    build_kernel()
    print("kernel built OK")
